# revision 29
# baseline (speedup 1.0000x reference)
"""Trainium2 Bass kernel for a dense transformer block (B=8, N=1024, C=768, H=12).

Sharding: data-parallel over batch -- one batch element per NeuronCore (8 cores),
weights replicated, no collectives.

v3: fp8e4 DoubleRow matmuls (256-deep contraction @ 0.5 cyc/row) for
QKV/V/proj/FC1/FC2/AV, bf16 row-packed score matmuls, bf16 transposes.

Rel-bias via softmax shift-invariance: subtract table[0]; below-diagonal
regions need nothing, the above-diagonal constant c2=table[128]-table[0] is a
rank-1 PE accumulate, and the 255-wide diagonal window is a PE identity-matmul
accumulate from a host-built [128,256] Toeplitz patch.

Residual stream carries a global x512 scale (x is pre-scaled on host;
layernorm is scale-invariant) so that proj (8*64) and fc2 (1*512) PSUMs add
straight into it; one tensor_scalar * (1/512) at the output.

proj and fc2 are "flipped" (stationary = activations, moving = weights, output
token-major) so one LDWEIGHTS serves several matmuls and no output transposes
are needed. fc1/fc2 weights ship as fp8 hi+lo pairs (lo rides in e4m3
subnormals) for a 2-term residual quantization. A post-schedule pass dedupes
back-to-back identical LDWEIGHTS, which the stack otherwise emits 1:1 per
matmul.
"""

import os

import numpy as np

B, N, C, H, D = 8, 1024, 768, 12, 64
NT = N // 128   # 8 token tiles
KT = C // 128   # 6 feature tiles
F1 = 4 * C      # 3072
RT = F1 // 128  # 24
EPS = 1e-5
WS = 64.0       # fp8 weight scale (qkv, proj, fc1)
RS = 512.0      # residual-stream scale; fc2 weight scale

LAST_RESULTS = None

_NC_CACHE = {}


def _dedupe_ldweights(nc):
    """Drop InstLdweights identical to the immediately-preceding one.

    The scheduler emits one Ldweights per matmul even when consecutive
    matmuls share the stationary operand; the duplicate loads are pure
    weight-port waste (256 cols @ 1.2 GHz each for DoubleRow)."""
    ndel = 0
    remap = {}
    for fn in nc.m.functions:
        for blk in fn.blocks:
            insts = list(blk.instructions)
            keep = []
            last_sig = None
            last_name = None
            changed = False
            for inst in insts:
                tn = type(inst).__name__
                if tn == "InstLdweights":
                    c = inst.concise()
                    sig = (
                        c.split("in=")[-1],
                        str(inst.perf_mode),
                        str(inst.is_transpose),
                        str(inst.tile_position),
                        tuple(sorted(inst.sync_dependency_names())),
                    )
                    if sig == last_sig and "wait:" not in c:
                        remap[inst.name] = last_name
                        ndel += 1
                        changed = True
                        continue
                    last_sig = sig
                    last_name = inst.name
                elif tn == "InstMatmult":
                    if inst.ldweights:
                        last_sig = None
                keep.append(inst)
            if changed:
                blk.instructions = keep
    if remap:
        for fn in nc.m.functions:
            for blk in fn.blocks:
                for inst in blk.instructions:
                    deps = set(inst.sync_dependency_names()) | set(
                        inst.nosync_dependency_names()
                    )
                    hits = deps & set(remap)
                    if hits:
                        inst.remap_dependency_names(
                            {old: remap[old] for old in hits}
                        )
    return ndel


def _build_nc(reps=1, has_vbias=False, has_pbias=False, has_fbias=False):
    from contextlib import ExitStack

    import concourse.bacc as bacc
    import concourse.tile as tile
    from concourse import masks, mybir

    f32 = mybir.dt.float32
    bf16 = mybir.dt.bfloat16
    f8 = mybir.dt.float8e4
    DR = mybir.MatmulPerfMode.DoubleRow

    AF = mybir.ActivationFunctionType
    AX = mybir.AxisListType
    OP = mybir.AluOpType

    nc = bacc.Bacc(
        "TRN2",
        target_bir_lowering=False,
        debug=False,
        enable_asserts=False,
        num_devices=8,
    )

    x_d = nc.dram_tensor("x", [N, C], f32, kind="ExternalInput").ap()
    wqkv_d = nc.dram_tensor("wqkv8", [3, 128, 2, 3 * C], f8, kind="ExternalInput").ap()
    bqkv_d = nc.dram_tensor("bqkv", [1, 3 * C], f32, kind="ExternalInput").ap()
    bv_d = nc.dram_tensor("bvrow", [1, C], bf16, kind="ExternalInput").ap()
    wproj_d = nc.dram_tensor("wproj8", [3, 128, 2, C], f8, kind="ExternalInput").ap()
    bproj_d = nc.dram_tensor("bprow", [1, C], bf16, kind="ExternalInput").ap()
    wfc1_d = nc.dram_tensor("wfc18", [6, 128, 2, F1], f8, kind="ExternalInput").ap()
    bfc1_d = nc.dram_tensor("bfc1", [1, F1], f32, kind="ExternalInput").ap()
    wfc2_d = nc.dram_tensor("wfc28", [24, 128, 2, C], f8, kind="ExternalInput").ap()
    bfc2_d = nc.dram_tensor("bfrow", [1, C], bf16, kind="ExternalInput").ap()
    patch_d = nc.dram_tensor("patch", [H, 128, 256], bf16, kind="ExternalInput").ap()
    c2_d = nc.dram_tensor("c2col", [1, H * 128], bf16, kind="ExternalInput").ap()
    sel_d = nc.dram_tensor("sel", [2, 128], bf16, kind="ExternalInput").ap()
    out_d = nc.dram_tensor("out", [N, C], f32, kind="ExternalOutput").ap()

    with tile.TileContext(nc) as tc, ExitStack() as ctx:
        cpool = ctx.enter_context(tc.tile_pool(name="const", bufs=1))
        identb = cpool.tile([128, 128], bf16, tag="identb")
        masks.make_identity(nc, identb[:])
        ones_bf = cpool.tile([1, 128], bf16, tag="onesb")
        nc.any.memset(ones_bf[:], 1.0)
        ones_row = cpool.tile([1, N], bf16, tag="onesrow")
        nc.any.memset(ones_row[:], 1.0)
        sel = cpool.tile([2, 128], bf16, tag="sel")
        nc.sync.dma_start(sel[:], sel_d[:])
        epsc = cpool.tile([128, 1], f32, tag="eps")
        nc.any.memset(epsc[:], EPS * RS * RS)
        patch = cpool.tile([128, H, 256], bf16, tag="patch")
        for h in range(H):
            nc.sync.dma_start(patch[:, h, :], patch_d[h])
        c2col = cpool.tile([1, H * 128], bf16, tag="c2col")
        nc.sync.dma_start(c2col[:], c2_d[:])
        bqkv_sb = cpool.tile([128, 18], f32, tag="bqkv")
        nc.sync.dma_start(bqkv_sb[:], bqkv_d[0].rearrange("(a p) -> p a", p=128))
        bv_row = cpool.tile([1, C], bf16, tag="bvrow")
        nc.sync.dma_start(bv_row[:], bv_d[:])
        bp_row = cpool.tile([1, C], bf16, tag="bprow")
        nc.sync.dma_start(bp_row[:], bproj_d[:])
        bfc1_sb = cpool.tile([128, RT], f32, tag="bfc1")
        nc.sync.dma_start(bfc1_sb[:], bfc1_d[0].rearrange("(a p) -> p a", p=128))
        bf_row = cpool.tile([1, C], bf16, tag="bfrow")
        nc.sync.dma_start(bf_row[:], bfc2_d[:])

        stat = ctx.enter_context(tc.tile_pool(name="stat", bufs=8))
        chain = ctx.enter_context(tc.tile_pool(name="chain", bufs=1))

        def layernorm(dst_ap, src_ap, scratch_ap):
            """dst = (src - mean(src)) * rsqrt(var(src) + eps').

            src carries the RS scale; eps' = EPS*RS^2 keeps the result equal
            to layernorm of src/RS. The square runs on GpSimd (SBUF-only)."""
            sums = stat.tile([128, 1], f32, tag="sums", name="sums")
            nc.vector.reduce_sum(sums[:], src_ap, axis=AX.X)
            mu = stat.tile([128, 1], f32, tag="mu", name="mu")
            nc.vector.tensor_scalar_mul(mu[:], sums[:], 1.0 / C)
            nc.gpsimd.tensor_mul(scratch_ap, src_ap, src_ap)
            ssq = stat.tile([128, 1], f32, tag="ssq", name="ssq")
            nc.vector.reduce_sum(ssq[:], scratch_ap, axis=AX.X)
            musq = stat.tile([128, 1], f32, tag="musq", name="musq")
            nc.vector.tensor_mul(musq[:], mu[:], mu[:])
            var = stat.tile([128, 1], f32, tag="var", name="var")
            nc.vector.tensor_scalar(
                var[:], ssq[:], 1.0 / C, musq[:], op0=OP.mult, op1=OP.subtract
            )
            sd = stat.tile([128, 1], f32, tag="sd", name="sd")
            nc.scalar.activation(sd[:], var[:], AF.Sqrt, bias=epsc[:])
            rstd = stat.tile([128, 1], f32, tag="rstd", name="rstd")
            nc.vector.reciprocal(rstd[:], sd[:])
            nmr = stat.tile([128, 1], f32, tag="nmr", name="nmr")
            nc.vector.tensor_scalar(
                nmr[:], mu[:], rstd[:], -1.0, op0=OP.mult, op1=OP.mult
            )
            nc.vector.tensor_scalar(
                dst_ap, src_ap, rstd[:], nmr[:], op0=OP.mult, op1=OP.add
            )

        for _rep in range(reps):
            xs = [chain.tile([128, C], f32, tag="x", bufs=NT, name=f"x{t}") for t in range(NT)]
            hT8 = chain.tile([128, KT, N], f8, tag="hT8", bufs=1, name="hT8")
            qkT = [
                chain.tile([128, N], bf16, tag="qkT", bufs=12, name=f"qkT{i}")
                for i in range(12)
            ]
            vaug = chain.tile([128, NT, H, 72], f8, tag="vaug", bufs=1, name="vaug")
            aT8 = chain.tile([128, KT, N], f8, tag="aT8", bufs=1, name="aT8")
            s_all = chain.tile([H, N], f32, tag="sall", bufs=1, name="sall")
            s8 = chain.tile([H, N], bf16, tag="s8", bufs=1, name="s8")

            # ---------------- phase A: load x (pre-scaled by RS on host),
            # LN1, transpose -> hT8 (fp8)
            with tc.tile_pool(name="psA", bufs=6, space="PSUM") as psA:
                for t in range(NT):
                    nc.sync.dma_start(xs[t][:], x_d[t * 128 : (t + 1) * 128, :])
                    h1 = chain.tile([128, C], bf16, tag="hln", bufs=3, name=f"h1_{t}")
                    scr = chain.tile([128, C], f32, tag="hscr", bufs=3, name=f"sc1_{t}")
                    layernorm(h1[:], xs[t][:], scr[:])
                    for ct in range(KT):
                        ps = psA.tile([128, 128], bf16, tag="tp", name="psa")
                        nc.tensor.transpose(
                            ps[:], h1[:, ct * 128 : (ct + 1) * 128], identb[:]
                        )
                        nc.vector.tensor_copy(
                            hT8[:, ct, t * 128 : (t + 1) * 128], ps[:]
                        )

            # ---------------- phase C: QKV via fp8 DoubleRow
            with tc.tile_pool(name="wqkv", bufs=3) as wq_pool:
                wq8 = []
                for cp in range(3):
                    wt = wq_pool.tile([128, 2, 3 * C], f8, tag="wq", name=f"wq{cp}")
                    nc.sync.dma_start(wt[:], wqkv_d[cp])
                    wq8.append(wt)
                with tc.tile_pool(name="psC", bufs=1, space="PSUM") as psC:
                    for jt in range(12):
                        pss = [
                            psC.tile([128, 512], f32, tag="ps", bufs=4, name=f"psc{qc}")
                            for qc in range(2)
                        ]
                        for cp in range(3):
                            for qc in range(2):
                                nc.tensor.matmul(
                                    pss[qc][:],
                                    wq8[cp][:, :, jt * 128 : (jt + 1) * 128],
                                    hT8[:, 2 * cp : 2 * cp + 2, qc * 512 : (qc + 1) * 512],
                                    start=(cp == 0),
                                    stop=(cp == 2),
                                    perf_mode=DR,
                                )
                        for qc in range(2):
                            nc.vector.tensor_scalar(
                                qkT[jt][:, qc * 512 : (qc + 1) * 512],
                                pss[qc][:],
                                1.0 / WS,
                                bqkv_sb[:, jt : jt + 1],
                                op0=OP.mult,
                                op1=OP.add,
                            )
                    # v token-major: lhsT = hT8 (stationary), rhs = wv
                    for t in range(NT):
                        psa = psC.tile([128, 512], f32, tag="psva", bufs=2, name="psva")
                        psb2 = psC.tile([128, 256], f32, tag="psvb", bufs=2, name="psvb")
                        for cp in range(3):
                            last = cp == 2 and not has_vbias
                            nc.tensor.matmul(
                                psa[:],
                                hT8[:, 2 * cp : 2 * cp + 2, t * 128 : (t + 1) * 128],
                                wq8[cp][:, :, 2 * C : 2 * C + 512],
                                start=(cp == 0),
                                stop=last,
                                perf_mode=DR,
                            )
                            nc.tensor.matmul(
                                psb2[:],
                                hT8[:, 2 * cp : 2 * cp + 2, t * 128 : (t + 1) * 128],
                                wq8[cp][:, :, 2 * C + 512 : 3 * C],
                                start=(cp == 0),
                                stop=last,
                                perf_mode=DR,
                            )
                        if has_vbias:
                            nc.tensor.matmul(
                                psa[:], ones_bf[:], bv_row[:, 0:512],
                                start=False, stop=True,
                            )
                            nc.tensor.matmul(
                                psb2[:], ones_bf[:], bv_row[:, 512:C],
                                start=False, stop=True,
                            )
                        nc.vector.tensor_scalar_mul(
                            vaug[:, t, 0:8, 0:64],
                            psa[:].rearrange("p (h e) -> p h e", e=64),
                            1.0 / WS,
                        )
                        nc.vector.tensor_scalar_mul(
                            vaug[:, t, 8:12, 0:64],
                            psb2[:].rearrange("p (h e) -> p h e", e=64),
                            1.0 / WS,
                        )
                nc.any.memset(vaug[:, :, :, 64:65], 1.0)
                nc.any.memset(vaug[:, :, :, 65:72], 0.0)

            # ---------------- phase D: attention scores + exp + AV
            with (
                tc.tile_pool(name="ptp", bufs=3) as pt_pool,
                tc.tile_pool(name="psS", bufs=2, space="PSUM") as psS,
                tc.tile_pool(name="psAV", bufs=4, space="PSUM") as psAV,
                tc.tile_pool(name="oddp", bufs=3) as oddp,
            ):
                for hp in range(KT):
                    pts = [
                        pt_pool.tile([128, NT, N], f8, tag="pt", name=f"pt{hp}_{o}")
                        for o in range(2)
                    ]
                    for kc in range(NT):
                        jA = max(0, kc * 128 - 63)
                        jB = min(N, kc * 128 + 192)
                        w0 = 63 if kc == 0 else 0
                        c2chunks = []
                        off = 0
                        while off < jA:
                            w = min(512, jA - off)
                            c2chunks.append((off, w))
                            off += w
                        for odd in range(2):
                            ro = odd * 64
                            h = 2 * hp + odd
                            ps = psS.tile([128, N], f32, tag="ps", name="pss")
                            for qc in range(2):
                                nc.tensor.matmul(
                                    ps[:, qc * 512 : (qc + 1) * 512],
                                    qkT[6 + hp][ro : ro + 64, kc * 128 : (kc + 1) * 128],
                                    qkT[hp][ro : ro + 64, qc * 512 : (qc + 1) * 512],
                                    start=True,
                                    stop=True,
                                )
                            for off, w in c2chunks:
                                nc.tensor.matmul(
                                    ps[:, off : off + w],
                                    c2col[:, h * 128 : (h + 1) * 128],
                                    ones_row[:, 0:w],
                                    start=False,
                                    stop=True,
                                    skip_group_check=True,
                                )
                            # diagonal-window rel-bias: accumulate the Toeplitz
                            # patch via identity matmul (split at PSUM banks)
                            segs = [
                                (a, b)
                                for a, b in ((jA, min(jB, 512)), (max(jA, 512), jB))
                                if b > a
                            ]
                            for a, b in segs:
                                nc.tensor.matmul(
                                    ps[:, a:b],
                                    identb[:],
                                    patch[:, h, w0 + (a - jA) : w0 + (b - jA)],
                                    start=False,
                                    stop=True,
                                    skip_group_check=True,
                                )
                            nc.scalar.activation(pts[odd][:, kc, :], ps[:], AF.Exp)
                    # AV with DoubleRow over kc pairs; ones col gives sums
                    for odd in range(2):
                        h = 2 * hp + odd
                        pavs = [
                            psAV.tile([72, 512], f32, tag="pav", bufs=4, name=f"pav{qc}")
                            for qc in range(2)
                        ]
                        for m in range(4):
                            for qc in range(2):
                                nc.tensor.matmul(
                                    pavs[qc][:],
                                    vaug[:, 2 * m : 2 * m + 2, h, :],
                                    pts[odd][:, 2 * m : 2 * m + 2, qc * 512 : (qc + 1) * 512],
                                    start=(m == 0),
                                    stop=(m == 3),
                                    perf_mode=DR,
                                )
                        for qc in range(2):
                            pav = pavs[qc]
                            srow = oddp.tile([128, 512], f32, tag="srow", name="srow")
                            nc.vector.tensor_copy(srow[64:65, :], pav[64:65, :])
                            nc.sync.dma_start(
                                s_all[h : h + 1, qc * 512 : (qc + 1) * 512],
                                srow[64:65, :],
                            )
                            if odd:
                                tmp = oddp.tile([64, 512], f8, tag="odd", name="avodd")
                                nc.vector.tensor_copy(tmp[:], pav[0:64, :])
                                nc.sync.dma_start(
                                    aT8[64:128, hp, qc * 512 : (qc + 1) * 512], tmp[:]
                                )
                            else:
                                nc.vector.tensor_copy(
                                    aT8[0:64, hp, qc * 512 : (qc + 1) * 512],
                                    pav[0:64, :],
                                )

            # ---------------- phase E: normalize aT8, then flipped proj
            # (token-major out) + residual into xs
            with tc.tile_pool(name="wpp", bufs=3) as wpp:
                wp8 = []
                for hp2 in range(3):
                    wt = wpp.tile([128, 2, C], f8, tag="wp", name=f"wp{hp2}")
                    nc.sync.dma_start(wt[:], wproj_d[hp2])
                    wp8.append(wt)
                with (
                    tc.tile_pool(name="st2p", bufs=4) as st2p,
                    tc.tile_pool(name="psNorm", bufs=2, space="PSUM") as psN,
                ):
                    nc.vector.reciprocal_approx_fast(s_all[:], s_all[:])
                    nc.vector.tensor_copy(s8[:], s_all[:])
                    for qc in range(2):
                        for hp in range(KT):
                            st2 = st2p.tile([2, 512], bf16, tag="stg", name="st2")
                            nc.sync.dma_start(
                                st2[:],
                                s8[2 * hp : 2 * hp + 2, qc * 512 : (qc + 1) * 512],
                            )
                            psb = psN.tile([128, 512], f32, tag="psn", name="psn")
                            nc.tensor.matmul(
                                psb[0:64, :], sel[:, 0:64], st2[:], start=True, stop=True
                            )
                            nc.tensor.matmul(
                                psb[64:128, :], sel[:, 64:128], st2[:],
                                start=True, stop=True,
                            )
                            nc.vector.tensor_mul(
                                aT8[:, hp, qc * 512 : (qc + 1) * 512],
                                aT8[:, hp, qc * 512 : (qc + 1) * 512],
                                psb[:],
                            )
                with tc.tile_pool(name="psP", bufs=1, space="PSUM") as psP:
                    for t in range(NT):
                        pja = psP.tile([128, 512], f32, tag="pja", bufs=2, name="pja")
                        pjb = psP.tile([128, 256], f32, tag="pjb", bufs=2, name="pjb")
                        for hp2 in range(3):
                            last = hp2 == 2 and not has_pbias
                            nc.tensor.matmul(
                                pja[:],
                                aT8[:, 2 * hp2 : 2 * hp2 + 2, t * 128 : (t + 1) * 128],
                                wp8[hp2][:, :, 0:512],
                                start=(hp2 == 0),
                                stop=last,
                                perf_mode=DR,
                            )
                            nc.tensor.matmul(
                                pjb[:],
                                aT8[:, 2 * hp2 : 2 * hp2 + 2, t * 128 : (t + 1) * 128],
                                wp8[hp2][:, :, 512:C],
                                start=(hp2 == 0),
                                stop=last,
                                perf_mode=DR,
                            )
                        if has_pbias:
                            nc.tensor.matmul(
                                pja[:], ones_bf[:], bp_row[:, 0:512],
                                start=False, stop=True,
                            )
                            nc.tensor.matmul(
                                pjb[:], ones_bf[:], bp_row[:, 512:C],
                                start=False, stop=True,
                            )
                        nc.vector.tensor_add(
                            xs[t][:, 0:512], xs[t][:, 0:512], pja[:]
                        )
                        nc.vector.tensor_add(
                            xs[t][:, 512:C], xs[t][:, 512:C], pjb[:]
                        )

            # ---------------- phase F: LN2 -> h2T8 (reuses hT8's buffer)
            h2T8 = chain.tile([128, KT, N], f8, tag="hT8", bufs=1, name="h2T8")
            with tc.tile_pool(name="psF", bufs=6, space="PSUM") as psF:
                for t in range(NT):
                    h2 = chain.tile([128, C], bf16, tag="hln", bufs=3, name=f"h2_{t}")
                    scr = chain.tile([128, C], f32, tag="hscr", bufs=3, name=f"sc2_{t}")
                    layernorm(h2[:], xs[t][:], scr[:])
                    for ct in range(KT):
                        ps = psF.tile([128, 128], bf16, tag="tp", name="psf")
                        nc.tensor.transpose(
                            ps[:], h2[:, ct * 128 : (ct + 1) * 128], identb[:]
                        )
                        nc.vector.tensor_copy(
                            h2T8[:, ct, t * 128 : (t + 1) * 128], ps[:]
                        )

            # ---------------- phase G: fc1 + gelu -> gr tiles
            # (reuse qkT's 12 x 2KB buffers: qkT is dead after phase D)
            gr = [
                chain.tile([128, 2, N], f8, tag="qkT", bufs=12, name=f"gr{rp}")
                for rp in range(12)
            ]
            with tc.tile_pool(name="w1p", bufs=6) as w1p:
                w18 = []
                for cp in range(6):  # 3 contraction pairs x {hi, lo}
                    wt = w1p.tile([128, 2, F1], f8, tag="w1", name=f"w1_{cp}")
                    nc.sync.dma_start(wt[:], wfc1_d[cp])
                    w18.append(wt)
                with tc.tile_pool(name="psG", bufs=2, space="PSUM") as psG:
                    for r in range(RT):
                        psg = psG.tile([128, N], f32, tag="psg", name="psg")
                        for cp in range(6):
                            for qc in range(2):
                                nc.tensor.matmul(
                                    psg[:, qc * 512 : (qc + 1) * 512],
                                    w18[cp][:, :, r * 128 : (r + 1) * 128],
                                    h2T8[:, 2 * (cp % 3) : 2 * (cp % 3) + 2, qc * 512 : (qc + 1) * 512],
                                    start=(cp == 0),
                                    stop=(cp == 5),
                                    perf_mode=DR,
                                )
                        nc.scalar.activation(
                            gr[r // 2][:, r % 2, :],
                            psg[:],
                            AF.Gelu,
                            bias=bfc1_sb[:, r : r + 1],
                            scale=1.0 / WS,
                        )

            # ---------------- phase H: flipped fc2 (token-major out,
            # stationary = gr slice shared by hi+lo weights) + residual + store
            with (
                tc.tile_pool(name="w2p", bufs=24) as w2p,
                tc.tile_pool(name="obp", bufs=3) as obp,
            ):
                w28 = []
                for rp in range(24):  # 12 contraction pairs x {hi, lo}
                    wt = w2p.tile([128, 2, C], f8, tag="w2", name=f"w2_{rp}")
                    nc.sync.dma_start(wt[:], wfc2_d[rp])
                    w28.append(wt)
                with tc.tile_pool(name="psO", bufs=1, space="PSUM") as psO:
                    for t in range(NT):
                        pca = psO.tile([128, 512], f32, tag="pca", bufs=2, name="pca")
                        pcb = psO.tile([128, 256], f32, tag="pcb", bufs=2, name="pcb")
                        for rp in range(12):
                            grs = gr[rp][:, :, t * 128 : (t + 1) * 128]
                            last = rp == 11 and not has_fbias
                            nc.tensor.matmul(
                                pca[:], grs, w28[rp][:, :, 0:512],
                                start=(rp == 0), stop=False, perf_mode=DR,
                            )
                            nc.tensor.matmul(
                                pcb[:], grs, w28[rp][:, :, 512:C],
                                start=(rp == 0), stop=False, perf_mode=DR,
                            )
                            nc.tensor.matmul(
                                pca[:], grs, w28[12 + rp][:, :, 0:512],
                                start=False, stop=last, perf_mode=DR,
                            )
                            nc.tensor.matmul(
                                pcb[:], grs, w28[12 + rp][:, :, 512:C],
                                start=False, stop=last, perf_mode=DR,
                            )
                        if has_fbias:
                            nc.tensor.matmul(
                                pca[:], ones_bf[:], bf_row[:, 0:512],
                                start=False, stop=True,
                            )
                            nc.tensor.matmul(
                                pcb[:], ones_bf[:], bf_row[:, 512:C],
                                start=False, stop=True,
                            )
                        nc.vector.tensor_add(
                            xs[t][:, 0:512], xs[t][:, 0:512], pca[:]
                        )
                        nc.vector.tensor_add(
                            xs[t][:, 512:C], xs[t][:, 512:C], pcb[:]
                        )
                        ob = obp.tile([128, C], f32, tag="ob", name="ob")
                        nc.vector.tensor_scalar_mul(ob[:], xs[t][:], 1.0 / RS)
                        nc.sync.dma_start(out_d[t * 128 : (t + 1) * 128, :], ob[:])

    ndel = _dedupe_ldweights(nc)
    if os.environ.get("LDW_DEBUG"):
        print(f"deduped {ndel} ldweights")
    nc.compile()
    return nc


def _get_nc(reps=1, flags=(False, False, False)):
    key = f"nc{reps}_{flags}"
    if key not in _NC_CACHE:
        _NC_CACHE[key] = _build_nc(reps, *flags)
    return _NC_CACHE[key]


def _host_prep(inputs):
    import ml_dtypes

    f8 = ml_dtypes.float8_e4m3
    bf = ml_dtypes.bfloat16

    inp = {k: np.asarray(v) for k, v in inputs.items()}
    # residual stream carries RS; layernorm is scale-invariant
    x = np.ascontiguousarray(inp["x"] * RS, dtype=np.float32)  # [8, 1024, 768]
    g1 = inp["ln1_g"].astype(np.float64)
    b1 = inp["ln1_b"].astype(np.float64)
    qkv_w = inp["qkv_w"].astype(np.float64)  # [2304, 768]
    Ws = qkv_w.copy()
    Ws[:C] *= D ** (-0.5)  # fold attention scale into Wq
    WqT = np.ascontiguousarray((Ws * g1[None, :]).T)  # [768, 2304]
    wqkv8 = np.ascontiguousarray(
        (WqT * WS).reshape(3, 2, 128, 3 * C).transpose(0, 2, 1, 3)
    ).astype(f8)
    bqkv = (Ws @ b1).astype(np.float32).reshape(1, 3 * C)
    has_vbias = bool(np.any(bqkv[0, 2 * C :] != 0))
    bv = (bqkv[0, 2 * C :] * WS).astype(bf).reshape(1, C)

    projT = inp["proj_w"].astype(np.float64).T  # [768, 768]
    wproj8 = np.ascontiguousarray(
        (projT * WS).reshape(3, 2, 128, C).transpose(0, 2, 1, 3)
    ).astype(f8)
    bproj = inp["proj_b"].astype(np.float64)
    has_pbias = bool(np.any(bproj != 0))
    bprow = (bproj * RS).astype(bf).reshape(1, C)

    g2 = inp["ln2_g"].astype(np.float64)
    b2 = inp["ln2_b"].astype(np.float64)
    fc1_w = inp["fc1_w"].astype(np.float64)  # [3072, 768]

    def split_hi_lo(w_scaled, nparts, width):
        """[K, M] scaled weights -> [2*nparts, 128, 2, M] fp8 hi then lo."""
        tiles = np.ascontiguousarray(
            w_scaled.reshape(nparts, 2, 128, width).transpose(0, 2, 1, 3)
        )
        hi = tiles.astype(f8)
        lo = (tiles - hi.astype(np.float64)).astype(f8)
        return np.concatenate([hi, lo], axis=0)

    w1T = (fc1_w * g2[None, :]).T  # [768, 3072]
    wfc18 = split_hi_lo(w1T * WS, 3, F1)
    bfc1 = (fc1_w @ b2 + inp["fc1_b"].astype(np.float64)).astype(np.float32)
    bfc1 = bfc1.reshape(1, F1)
    w2T = inp["fc2_w"].astype(np.float64).T  # [3072, 768]
    wfc28 = split_hi_lo(w2T * RS, 12, C)
    bfc2 = inp["fc2_b"].astype(np.float64)
    has_fbias = bool(np.any(bfc2 != 0))
    bfrow = (bfc2 * RS).astype(bf).reshape(1, C)

    tab = inp["rel_table"].astype(np.float64)  # [129, 12]
    p_i = np.arange(128)[:, None]
    w_i = np.arange(256)[None, :]
    idx = np.clip(p_i - w_i + 127, 0, 128)
    patch = np.ascontiguousarray(
        (tab[idx, :] - tab[0, :]).transpose(2, 0, 1)
    ).astype(bf)  # [12, 128, 256]
    c2 = (tab[128, :] - tab[0, :]).astype(np.float32)  # [12]
    c2col = np.repeat(c2[:, None], 128, axis=1).reshape(1, H * 128).astype(bf)

    selm = np.zeros((2, 128), np.float32)
    selm[0, 0:64] = 8.0
    selm[1, 64:128] = 8.0
    selm = selm.astype(bf)

    shared = {
        "sel": selm,
        "wqkv8": wqkv8,
        "bqkv": bqkv,
        "bvrow": bv,
        "wproj8": wproj8,
        "bprow": bprow,
        "wfc18": wfc18,
        "bfc1": bfc1,
        "wfc28": wfc28,
        "bfrow": bfrow,
        "patch": patch,
        "c2col": c2col,
    }
    in_maps = [{"x": np.ascontiguousarray(x[c]), **shared} for c in range(B)]
    return in_maps, (has_vbias, has_pbias, has_fbias)


def _make_runner(reps=1, flags=(False, False, False)):
    import jax
    from jax.experimental.shard_map import shard_map
    from jax.sharding import Mesh, NamedSharding, PartitionSpec

    from concourse import bass2jax, mybir

    nc = _get_nc(reps, flags)
    bass2jax.install_neuronx_cc_hook()

    partition_name = nc.partition_id_tensor.name if nc.partition_id_tensor else None
    in_names, out_names, out_avals, zero_outs = [], [], [], []
    for alloc in nc.m.functions[0].allocations:
        if not isinstance(alloc, mybir.MemoryLocationSet):
            continue
        name = alloc.memorylocations[0].name
        if alloc.kind == "ExternalInput":
            if name != partition_name:
                in_names.append(name)
        elif alloc.kind == "ExternalOutput":
            out_names.append(name)
            shape = tuple(alloc.tensor_shape)
            dtype = mybir.dt.np(alloc.dtype)
            out_avals.append(jax.core.ShapedArray(shape, dtype))
            zero_outs.append(np.zeros(shape, dtype))
    n_params = len(in_names)
    all_names = tuple(in_names) + tuple(out_names)
    if partition_name is not None:
        all_names = all_names + (partition_name,)
    donate = tuple(range(n_params, n_params + len(out_names)))

    def _body(*args):
        operands = list(args)
        if partition_name is not None:
            operands.append(bass2jax.partition_id_tensor())
        outs = bass2jax._bass_exec_p.bind(
            *operands,
            out_avals=tuple(out_avals),
            in_names=all_names,
            out_names=tuple(out_names),
            lowering_input_output_aliases=(),
            sim_require_finite=True,
            sim_require_nnan=True,
            nc=nc,
        )
        return tuple(outs)

    def _body_k(k):
        def body(*args):
            ins = list(args[:n_params])
            outs = list(args[n_params:])
            for _ in range(k):
                outs = list(_body(*ins, *outs))
            return tuple(outs)

        return body

    devices = jax.devices()[:B]
    mesh = Mesh(np.asarray(devices), ("core",))
    in_specs = (PartitionSpec("core"),) * (n_params + len(out_names))
    out_specs = (PartitionSpec("core"),) * len(out_names)

    def make_fn(k):
        return jax.jit(
            shard_map(
                _body_k(k),
                mesh=mesh,
                in_specs=in_specs,
                out_specs=out_specs,
                check_rep=False,
            ),
            donate_argnums=donate,
            keep_unused=True,
        )

    sharding = NamedSharding(mesh, PartitionSpec("core"))
    return make_fn, in_names, out_names, zero_outs, sharding


def _get_runner(reps=1, flags=(False, False, False)):
    key = f"runner{reps}_{flags}"
    if key not in _NC_CACHE:
        _NC_CACHE[key] = _make_runner(reps, flags)
    return _NC_CACHE[key]


LAST_BENCH = None


def kernel(**inputs):
    global LAST_BENCH
    import time

    import jax

    in_maps, flags = _host_prep(inputs)
    make_fn, in_names, out_names, zero_outs, sharding = _get_runner(1, flags)
    concat_in = [
        np.concatenate([np.asarray(in_maps[c][n]) for c in range(B)], axis=0)
        for n in in_names
    ]
    concat_zeros = [
        np.zeros((B * z.shape[0], *z.shape[1:]), z.dtype) for z in zero_outs
    ]
    fn1 = make_fn(1)
    dev_in = [jax.device_put(a, sharding) for a in concat_in]
    outs = fn1(*dev_in, *concat_zeros)
    jax.block_until_ready(outs)
    result = np.asarray(outs[0]).reshape(B, N, C).astype(np.float32)

    iters = int(os.environ.get("BENCH_ITERS", "0"))
    if iters > 0:
        o = fn1(*dev_in, *outs)  # warm
        jax.block_until_ready(o)
        times = []
        for _ in range(iters):
            t0 = time.perf_counter()
            o = fn1(*dev_in, *o)
            jax.block_until_ready(o)
            times.append(time.perf_counter() - t0)
        overhead = _bench_overhead()
        t_min = float(np.min(times))
        t_med = float(np.median(times))
        LAST_BENCH = {
            "per_iter_ns": max(t_min - overhead, 0.0) * 1e9,
            "call_min_ns": t_min * 1e9,
            "call_med_ns": t_med * 1e9,
            "overhead_ns": overhead * 1e9,
            "iters": iters,
        }
    return result


def _bench_overhead():
    """Per-call dispatch overhead, measured with a trivial 1-DMA kernel."""
    import time

    import jax
    from jax.experimental.shard_map import shard_map
    from jax.sharding import Mesh, PartitionSpec

    import concourse.bacc as bacc
    import concourse.tile as tile
    from concourse import bass2jax, mybir

    if "tiny" not in _NC_CACHE:
        f32 = mybir.dt.float32
        nc = bacc.Bacc(
            "TRN2",
            target_bir_lowering=False,
            debug=False,
            enable_asserts=False,
            num_devices=8,
        )
        xi = nc.dram_tensor("ti", [128, 128], f32, kind="ExternalInput").ap()
        xo = nc.dram_tensor("to", [128, 128], f32, kind="ExternalOutput").ap()
        with tile.TileContext(nc) as tc:
            with tc.tile_pool(name="p", bufs=1) as p:
                t = p.tile([128, 128], f32, tag="t", name="t")
                nc.sync.dma_start(t[:], xi[:])
                nc.sync.dma_start(xo[:], t[:])
        nc.compile()

        partition_name = nc.partition_id_tensor.name if nc.partition_id_tensor else None
        all_names = ["ti", "to"]
        if partition_name is not None:
            all_names.append(partition_name)
        out_avals = [jax.core.ShapedArray((128, 128), np.float32)]

        def _tbody(*args):
            operands = list(args)
            if partition_name is not None:
                operands.append(bass2jax.partition_id_tensor())
            return tuple(
                bass2jax._bass_exec_p.bind(
                    *operands,
                    out_avals=tuple(out_avals),
                    in_names=tuple(all_names),
                    out_names=("to",),
                    lowering_input_output_aliases=(),
                    sim_require_finite=True,
                    sim_require_nnan=True,
                    nc=nc,
                )
            )

        devices = jax.devices()[:B]
        mesh = Mesh(np.asarray(devices), ("core",))
        tfn = jax.jit(
            shard_map(
                _tbody,
                mesh=mesh,
                in_specs=(PartitionSpec("core"),) * 2,
                out_specs=(PartitionSpec("core"),),
                check_rep=False,
            ),
            donate_argnums=(1,),
            keep_unused=True,
        )
        _NC_CACHE["tiny"] = tfn

    tfn = _NC_CACHE["tiny"]
    ti = np.zeros((B * 128, 128), np.float32)
    o = tfn(ti, np.zeros((B * 128, 128), np.float32))
    jax.block_until_ready(o)
    times = []
    for _ in range(30):
        t0 = time.perf_counter()
        o = tfn(ti, *([o] if not isinstance(o, tuple) else list(o)))
        jax.block_until_ready(o)
        times.append(time.perf_counter() - t0)
    return float(np.min(times))


# revision 33
# speedup vs baseline: 1.0089x; 1.0089x over previous
"""Trainium2 Bass kernel for a dense transformer block (B=8, N=1024, C=768, H=12).

Sharding: data-parallel over batch -- one batch element per NeuronCore (8 cores),
weights replicated, no collectives.

v3: fp8e4 DoubleRow matmuls (256-deep contraction @ 0.5 cyc/row) for
QKV/V/proj/FC1/FC2/AV, bf16 row-packed score matmuls, bf16 transposes.

Rel-bias via softmax shift-invariance: subtract table[0]; below-diagonal
regions need nothing, the above-diagonal constant c2=table[128]-table[0] is a
rank-1 PE accumulate, and the 255-wide diagonal window is a PE identity-matmul
accumulate from a host-built [128,256] Toeplitz patch.

Residual stream carries a global x512 scale (x is pre-scaled on host;
layernorm is scale-invariant) so that proj (8*64) and fc2 (1*512) PSUMs add
straight into it; one tensor_scalar * (1/512) at the output.

proj and fc2 are "flipped" (stationary = activations, moving = weights, output
token-major) so one LDWEIGHTS serves several matmuls and no output transposes
are needed. fc1/fc2 weights ship as fp8 hi+lo pairs (lo rides in e4m3
subnormals) for a 2-term residual quantization. A post-schedule pass dedupes
back-to-back identical LDWEIGHTS, which the stack otherwise emits 1:1 per
matmul.
"""

import os

import numpy as np

B, N, C, H, D = 8, 1024, 768, 12, 64
NT = N // 128   # 8 token tiles
KT = C // 128   # 6 feature tiles
F1 = 4 * C      # 3072
RT = F1 // 128  # 24
EPS = 1e-5
WS = 64.0       # fp8 weight scale (qkv, proj, fc1)
RS = 512.0      # residual-stream scale; fc2 weight scale

LAST_RESULTS = None

_NC_CACHE = {}


def _dedupe_ldweights(nc):
    """Drop InstLdweights identical to the immediately-preceding one.

    The scheduler emits one Ldweights per matmul even when consecutive
    matmuls share the stationary operand; the duplicate loads are pure
    weight-port waste (256 cols @ 1.2 GHz each for DoubleRow)."""
    ndel = 0
    remap = {}
    for fn in nc.m.functions:
        for blk in fn.blocks:
            insts = list(blk.instructions)
            keep = []
            last_sig = None
            last_name = None
            changed = False
            for inst in insts:
                tn = type(inst).__name__
                if tn == "InstLdweights":
                    c = inst.concise()
                    sig = (
                        c.split("in=")[-1],
                        str(inst.perf_mode),
                        str(inst.is_transpose),
                        str(inst.tile_position),
                        tuple(sorted(inst.sync_dependency_names())),
                    )
                    if sig == last_sig and "wait:" not in c:
                        remap[inst.name] = last_name
                        ndel += 1
                        changed = True
                        continue
                    last_sig = sig
                    last_name = inst.name
                elif tn == "InstMatmult":
                    if inst.ldweights:
                        last_sig = None
                keep.append(inst)
            if changed:
                blk.instructions = keep
    if remap:
        for fn in nc.m.functions:
            for blk in fn.blocks:
                for inst in blk.instructions:
                    deps = set(inst.sync_dependency_names()) | set(
                        inst.nosync_dependency_names()
                    )
                    hits = deps & set(remap)
                    if hits:
                        inst.remap_dependency_names(
                            {old: remap[old] for old in hits}
                        )
    return ndel


def _build_nc(reps=1, has_vbias=False, has_pbias=False, has_fbias=False):
    from contextlib import ExitStack

    import concourse.bacc as bacc
    import concourse.tile as tile
    from concourse import masks, mybir

    f32 = mybir.dt.float32
    bf16 = mybir.dt.bfloat16
    f8 = mybir.dt.float8e4
    DR = mybir.MatmulPerfMode.DoubleRow

    AF = mybir.ActivationFunctionType
    AX = mybir.AxisListType
    OP = mybir.AluOpType

    nc = bacc.Bacc(
        "TRN2",
        target_bir_lowering=False,
        debug=False,
        enable_asserts=False,
        num_devices=8,
    )

    x_d = nc.dram_tensor("x", [N, C], f32, kind="ExternalInput").ap()
    wqkv_d = nc.dram_tensor("wqkv8", [3, 128, 2, 3 * C], f8, kind="ExternalInput").ap()
    bqkv_d = nc.dram_tensor("bqkv", [1, 3 * C], f32, kind="ExternalInput").ap()
    bv_d = nc.dram_tensor("bvrow", [1, C], bf16, kind="ExternalInput").ap()
    wproj_d = nc.dram_tensor("wproj8", [3, 128, 2, C], f8, kind="ExternalInput").ap()
    bproj_d = nc.dram_tensor("bprow", [1, C], bf16, kind="ExternalInput").ap()
    wfc1_d = nc.dram_tensor("wfc18", [6, 128, 2, F1], f8, kind="ExternalInput").ap()
    bfc1_d = nc.dram_tensor("bfc1", [1, F1], f32, kind="ExternalInput").ap()
    wfc2_d = nc.dram_tensor("wfc28", [24, 128, 2, C], f8, kind="ExternalInput").ap()
    bfc2_d = nc.dram_tensor("bfrow", [1, C], bf16, kind="ExternalInput").ap()
    patch_d = nc.dram_tensor("patch", [H, 128, 256], bf16, kind="ExternalInput").ap()
    c2_d = nc.dram_tensor("c2col", [1, H * 128], bf16, kind="ExternalInput").ap()
    sel_d = nc.dram_tensor("sel", [2, 128], bf16, kind="ExternalInput").ap()
    out_d = nc.dram_tensor("out", [N, C], f32, kind="ExternalOutput").ap()

    with tile.TileContext(nc) as tc, ExitStack() as ctx:
        cpool = ctx.enter_context(tc.tile_pool(name="const", bufs=1))
        identb = cpool.tile([128, 128], bf16, tag="identb")
        masks.make_identity(nc, identb[:])
        ones_bf = cpool.tile([1, 128], bf16, tag="onesb")
        nc.any.memset(ones_bf[:], 1.0)
        ones_row = cpool.tile([1, N], bf16, tag="onesrow")
        nc.any.memset(ones_row[:], 1.0)
        sel = cpool.tile([2, 128], bf16, tag="sel")
        nc.sync.dma_start(sel[:], sel_d[:])
        epsc = cpool.tile([128, 1], f32, tag="eps")
        nc.any.memset(epsc[:], EPS * RS * RS)
        patch = cpool.tile([128, H, 256], bf16, tag="patch")
        for h in range(H):
            nc.sync.dma_start(patch[:, h, :], patch_d[h])
        c2col = cpool.tile([1, H * 128], bf16, tag="c2col")
        nc.sync.dma_start(c2col[:], c2_d[:])
        bqkv_sb = cpool.tile([128, 18], f32, tag="bqkv")
        nc.sync.dma_start(bqkv_sb[:], bqkv_d[0].rearrange("(a p) -> p a", p=128))
        bv_row = cpool.tile([1, C], bf16, tag="bvrow")
        nc.sync.dma_start(bv_row[:], bv_d[:])
        bp_row = cpool.tile([1, C], bf16, tag="bprow")
        nc.sync.dma_start(bp_row[:], bproj_d[:])
        bfc1_sb = cpool.tile([128, RT], f32, tag="bfc1")
        nc.sync.dma_start(bfc1_sb[:], bfc1_d[0].rearrange("(a p) -> p a", p=128))
        bf_row = cpool.tile([1, C], bf16, tag="bfrow")
        nc.sync.dma_start(bf_row[:], bfc2_d[:])

        stat = ctx.enter_context(tc.tile_pool(name="stat", bufs=8))
        chain = ctx.enter_context(tc.tile_pool(name="chain", bufs=1))

        def layernorm(dst_ap, src_ap, scratch_ap):
            """dst = (src - mean(src)) * rsqrt(var(src) + eps').

            src carries the RS scale; eps' = EPS*RS^2 keeps the result equal
            to layernorm of src/RS. The square runs on GpSimd (SBUF-only)."""
            sums = stat.tile([128, 1], f32, tag="sums", name="sums")
            nc.vector.reduce_sum(sums[:], src_ap, axis=AX.X)
            mu = stat.tile([128, 1], f32, tag="mu", name="mu")
            nc.vector.tensor_scalar_mul(mu[:], sums[:], 1.0 / C)
            nc.gpsimd.tensor_mul(scratch_ap, src_ap, src_ap)
            ssq = stat.tile([128, 1], f32, tag="ssq", name="ssq")
            nc.vector.reduce_sum(ssq[:], scratch_ap, axis=AX.X)
            musq = stat.tile([128, 1], f32, tag="musq", name="musq")
            nc.vector.tensor_mul(musq[:], mu[:], mu[:])
            var = stat.tile([128, 1], f32, tag="var", name="var")
            nc.vector.tensor_scalar(
                var[:], ssq[:], 1.0 / C, musq[:], op0=OP.mult, op1=OP.subtract
            )
            sd = stat.tile([128, 1], f32, tag="sd", name="sd")
            nc.scalar.activation(sd[:], var[:], AF.Sqrt, bias=epsc[:])
            rstd = stat.tile([128, 1], f32, tag="rstd", name="rstd")
            nc.vector.reciprocal(rstd[:], sd[:])
            nmr = stat.tile([128, 1], f32, tag="nmr", name="nmr")
            nc.vector.tensor_scalar(
                nmr[:], mu[:], rstd[:], -1.0, op0=OP.mult, op1=OP.mult
            )
            nc.vector.tensor_scalar(
                dst_ap, src_ap, rstd[:], nmr[:], op0=OP.mult, op1=OP.add
            )

        for _rep in range(reps):
            xs = [chain.tile([128, C], f32, tag="x", bufs=NT, name=f"x{t}") for t in range(NT)]
            hT8 = chain.tile([128, KT, N], f8, tag="hT8", bufs=1, name="hT8")
            qkT = [
                chain.tile([128, N], bf16, tag="qkT", bufs=12, name=f"qkT{i}")
                for i in range(12)
            ]
            vaug = chain.tile([128, NT, H, 72], f8, tag="vaug", bufs=1, name="vaug")
            aT8 = chain.tile([128, KT, N], f8, tag="aT8", bufs=1, name="aT8")
            s_all = chain.tile([H, N], f32, tag="sall", bufs=1, name="sall")
            s8 = chain.tile([H, N], bf16, tag="s8", bufs=1, name="s8")

            # ---------------- phase A: load x (pre-scaled by RS on host),
            # LN1, transpose -> hT8 (fp8)
            with tc.tile_pool(name="psA", bufs=6, space="PSUM") as psA:
                for t in range(NT):
                    nc.sync.dma_start(xs[t][:], x_d[t * 128 : (t + 1) * 128, :])
                    h1 = chain.tile([128, C], bf16, tag="hln", bufs=3, name=f"h1_{t}")
                    scr = chain.tile([128, C], f32, tag="hscr", bufs=3, name=f"sc1_{t}")
                    layernorm(h1[:], xs[t][:], scr[:])
                    for ct in range(KT):
                        ps = psA.tile([128, 128], bf16, tag="tp", name="psa")
                        nc.tensor.transpose(
                            ps[:], h1[:, ct * 128 : (ct + 1) * 128], identb[:]
                        )
                        nc.vector.tensor_copy(
                            hT8[:, ct, t * 128 : (t + 1) * 128], ps[:]
                        )

            # ---------------- phase C: QKV via fp8 DoubleRow
            with tc.tile_pool(name="wqkv", bufs=3) as wq_pool:
                wq8 = []
                for cp in range(3):
                    wt = wq_pool.tile([128, 2, 3 * C], f8, tag="wq", name=f"wq{cp}")
                    nc.sync.dma_start(wt[:], wqkv_d[cp])
                    wq8.append(wt)
                with tc.tile_pool(name="psC", bufs=1, space="PSUM") as psC:
                    for jt in range(12):
                        pss = [
                            psC.tile([128, 512], f32, tag="ps", bufs=4, name=f"psc{qc}")
                            for qc in range(2)
                        ]
                        for cp in range(3):
                            for qc in range(2):
                                nc.tensor.matmul(
                                    pss[qc][:],
                                    wq8[cp][:, :, jt * 128 : (jt + 1) * 128],
                                    hT8[:, 2 * cp : 2 * cp + 2, qc * 512 : (qc + 1) * 512],
                                    start=(cp == 0),
                                    stop=(cp == 2),
                                    perf_mode=DR,
                                )
                        for qc in range(2):
                            nc.vector.tensor_scalar(
                                qkT[jt][:, qc * 512 : (qc + 1) * 512],
                                pss[qc][:],
                                1.0 / WS,
                                bqkv_sb[:, jt : jt + 1],
                                op0=OP.mult,
                                op1=OP.add,
                            )
                    # v token-major: lhsT = hT8 (stationary), rhs = wv
                    for t in range(NT):
                        psa = psC.tile([128, 512], f32, tag="psva", bufs=2, name="psva")
                        psb2 = psC.tile([128, 256], f32, tag="psvb", bufs=2, name="psvb")
                        for cp in range(3):
                            last = cp == 2 and not has_vbias
                            nc.tensor.matmul(
                                psa[:],
                                hT8[:, 2 * cp : 2 * cp + 2, t * 128 : (t + 1) * 128],
                                wq8[cp][:, :, 2 * C : 2 * C + 512],
                                start=(cp == 0),
                                stop=last,
                                perf_mode=DR,
                            )
                            nc.tensor.matmul(
                                psb2[:],
                                hT8[:, 2 * cp : 2 * cp + 2, t * 128 : (t + 1) * 128],
                                wq8[cp][:, :, 2 * C + 512 : 3 * C],
                                start=(cp == 0),
                                stop=last,
                                perf_mode=DR,
                            )
                        if has_vbias:
                            nc.tensor.matmul(
                                psa[:], ones_bf[:], bv_row[:, 0:512],
                                start=False, stop=True,
                            )
                            nc.tensor.matmul(
                                psb2[:], ones_bf[:], bv_row[:, 512:C],
                                start=False, stop=True,
                            )
                        nc.vector.tensor_scalar_mul(
                            vaug[:, t, 0:8, 0:64],
                            psa[:].rearrange("p (h e) -> p h e", e=64),
                            1.0 / WS,
                        )
                        nc.vector.tensor_scalar_mul(
                            vaug[:, t, 8:12, 0:64],
                            psb2[:].rearrange("p (h e) -> p h e", e=64),
                            1.0 / WS,
                        )
                nc.any.memset(vaug[:, :, :, 64:65], 1.0)
                nc.any.memset(vaug[:, :, :, 65:72], 0.0)

            # ---------------- phase D: attention scores + exp + AV
            with (
                tc.tile_pool(name="ptp", bufs=3) as pt_pool,
                tc.tile_pool(name="psS", bufs=2, space="PSUM") as psS,
                tc.tile_pool(name="psAV", bufs=4, space="PSUM") as psAV,
                tc.tile_pool(name="oddp", bufs=3) as oddp,
            ):
                for hp in range(KT):
                    pts = [
                        pt_pool.tile([128, NT, N], f8, tag="pt", name=f"pt{hp}_{o}")
                        for o in range(2)
                    ]
                    for kc in range(NT):
                        jA = max(0, kc * 128 - 63)
                        jB = min(N, kc * 128 + 192)
                        w0 = 63 if kc == 0 else 0
                        c2chunks = []
                        off = 0
                        while off < jA:
                            w = min(512, jA - off)
                            c2chunks.append((off, w))
                            off += w
                        # even/odd score matmuls adjacent: they target disjoint
                        # PE row groups (base partition 0 vs 64) and overlap
                        pse_o = [
                            psS.tile([128, N], f32, tag="ps", bufs=3, name=f"pss{o}")
                            for o in range(2)
                        ]
                        for qc in range(2):
                            for odd in range(2):
                                ro = odd * 64
                                nc.tensor.matmul(
                                    pse_o[odd][:, qc * 512 : (qc + 1) * 512],
                                    qkT[6 + hp][ro : ro + 64, kc * 128 : (kc + 1) * 128],
                                    qkT[hp][ro : ro + 64, qc * 512 : (qc + 1) * 512],
                                    start=True,
                                    stop=True,
                                )
                        for odd in range(2):
                            h = 2 * hp + odd
                            ps = pse_o[odd]
                            for off, w in c2chunks:
                                nc.tensor.matmul(
                                    ps[:, off : off + w],
                                    c2col[:, h * 128 : (h + 1) * 128],
                                    ones_row[:, 0:w],
                                    start=False,
                                    stop=True,
                                    skip_group_check=True,
                                )
                            # diagonal-window rel-bias: accumulate the Toeplitz
                            # patch via identity matmul (split at PSUM banks)
                            segs = [
                                (a, b)
                                for a, b in ((jA, min(jB, 512)), (max(jA, 512), jB))
                                if b > a
                            ]
                            for a, b in segs:
                                nc.tensor.matmul(
                                    ps[:, a:b],
                                    identb[:],
                                    patch[:, h, w0 + (a - jA) : w0 + (b - jA)],
                                    start=False,
                                    stop=True,
                                    skip_group_check=True,
                                )
                            nc.scalar.activation(pts[odd][:, kc, :], ps[:], AF.Exp)
                    # AV with DoubleRow over kc pairs; ones col gives sums
                    for odd in range(2):
                        h = 2 * hp + odd
                        pavs = [
                            psAV.tile([72, 512], f32, tag="pav", bufs=2, name=f"pav{qc}")
                            for qc in range(2)
                        ]
                        for m in range(4):
                            for qc in range(2):
                                nc.tensor.matmul(
                                    pavs[qc][:],
                                    vaug[:, 2 * m : 2 * m + 2, h, :],
                                    pts[odd][:, 2 * m : 2 * m + 2, qc * 512 : (qc + 1) * 512],
                                    start=(m == 0),
                                    stop=(m == 3),
                                    perf_mode=DR,
                                )
                        for qc in range(2):
                            pav = pavs[qc]
                            srow = oddp.tile([128, 512], f32, tag="srow", name="srow")
                            nc.vector.tensor_copy(srow[64:65, :], pav[64:65, :])
                            nc.sync.dma_start(
                                s_all[h : h + 1, qc * 512 : (qc + 1) * 512],
                                srow[64:65, :],
                            )
                            if odd:
                                tmp = oddp.tile([64, 512], f8, tag="odd", name="avodd")
                                nc.vector.tensor_copy(tmp[:], pav[0:64, :])
                                nc.sync.dma_start(
                                    aT8[64:128, hp, qc * 512 : (qc + 1) * 512], tmp[:]
                                )
                            else:
                                nc.vector.tensor_copy(
                                    aT8[0:64, hp, qc * 512 : (qc + 1) * 512],
                                    pav[0:64, :],
                                )

            # ---------------- phase E: normalize aT8, then flipped proj
            # (token-major out) + residual into xs
            with tc.tile_pool(name="wpp", bufs=3) as wpp:
                wp8 = []
                for hp2 in range(3):
                    wt = wpp.tile([128, 2, C], f8, tag="wp", name=f"wp{hp2}")
                    nc.sync.dma_start(wt[:], wproj_d[hp2])
                    wp8.append(wt)
                with (
                    tc.tile_pool(name="st2p", bufs=4) as st2p,
                    tc.tile_pool(name="psNorm", bufs=2, space="PSUM") as psN,
                ):
                    nc.vector.reciprocal_approx_fast(s_all[:], s_all[:])
                    nc.vector.tensor_copy(s8[:], s_all[:])
                    for qc in range(2):
                        for hp in range(KT):
                            st2 = st2p.tile([2, 512], bf16, tag="stg", name="st2")
                            nc.sync.dma_start(
                                st2[:],
                                s8[2 * hp : 2 * hp + 2, qc * 512 : (qc + 1) * 512],
                            )
                            psb = psN.tile([128, 512], f32, tag="psn", name="psn")
                            nc.tensor.matmul(
                                psb[0:64, :], sel[:, 0:64], st2[:], start=True, stop=True
                            )
                            nc.tensor.matmul(
                                psb[64:128, :], sel[:, 64:128], st2[:],
                                start=True, stop=True,
                            )
                            nc.vector.tensor_mul(
                                aT8[:, hp, qc * 512 : (qc + 1) * 512],
                                aT8[:, hp, qc * 512 : (qc + 1) * 512],
                                psb[:],
                            )
                with tc.tile_pool(name="psP", bufs=1, space="PSUM") as psP:
                    for t in range(NT):
                        pja = psP.tile([128, 512], f32, tag="pja", bufs=2, name="pja")
                        pjb = psP.tile([128, 256], f32, tag="pjb", bufs=2, name="pjb")
                        for hp2 in range(3):
                            last = hp2 == 2 and not has_pbias
                            nc.tensor.matmul(
                                pja[:],
                                aT8[:, 2 * hp2 : 2 * hp2 + 2, t * 128 : (t + 1) * 128],
                                wp8[hp2][:, :, 0:512],
                                start=(hp2 == 0),
                                stop=last,
                                perf_mode=DR,
                            )
                            nc.tensor.matmul(
                                pjb[:],
                                aT8[:, 2 * hp2 : 2 * hp2 + 2, t * 128 : (t + 1) * 128],
                                wp8[hp2][:, :, 512:C],
                                start=(hp2 == 0),
                                stop=last,
                                perf_mode=DR,
                            )
                        if has_pbias:
                            nc.tensor.matmul(
                                pja[:], ones_bf[:], bp_row[:, 0:512],
                                start=False, stop=True,
                            )
                            nc.tensor.matmul(
                                pjb[:], ones_bf[:], bp_row[:, 512:C],
                                start=False, stop=True,
                            )
                        nc.vector.tensor_add(
                            xs[t][:, 0:512], xs[t][:, 0:512], pja[:]
                        )
                        nc.vector.tensor_add(
                            xs[t][:, 512:C], xs[t][:, 512:C], pjb[:]
                        )

            # ---------------- phase F: LN2 -> h2T8 (reuses hT8's buffer)
            h2T8 = chain.tile([128, KT, N], f8, tag="hT8", bufs=1, name="h2T8")
            with tc.tile_pool(name="psF", bufs=6, space="PSUM") as psF:
                for t in range(NT):
                    h2 = chain.tile([128, C], bf16, tag="hln", bufs=3, name=f"h2_{t}")
                    scr = chain.tile([128, C], f32, tag="hscr", bufs=3, name=f"sc2_{t}")
                    layernorm(h2[:], xs[t][:], scr[:])
                    for ct in range(KT):
                        ps = psF.tile([128, 128], bf16, tag="tp", name="psf")
                        nc.tensor.transpose(
                            ps[:], h2[:, ct * 128 : (ct + 1) * 128], identb[:]
                        )
                        nc.vector.tensor_copy(
                            h2T8[:, ct, t * 128 : (t + 1) * 128], ps[:]
                        )

            # ---------------- phase G: fc1 + gelu -> gr tiles
            # (reuse qkT's 12 x 2KB buffers: qkT is dead after phase D)
            gr = [
                chain.tile([128, 2, N], f8, tag="qkT", bufs=12, name=f"gr{rp}")
                for rp in range(12)
            ]
            with tc.tile_pool(name="w1p", bufs=6) as w1p:
                w18 = []
                for cp in range(6):  # 3 contraction pairs x {hi, lo}
                    wt = w1p.tile([128, 2, F1], f8, tag="w1", name=f"w1_{cp}")
                    nc.sync.dma_start(wt[:], wfc1_d[cp])
                    w18.append(wt)
                with tc.tile_pool(name="psG", bufs=2, space="PSUM") as psG:
                    for r in range(RT):
                        psg = psG.tile([128, N], f32, tag="psg", name="psg")
                        for cp in range(6):
                            for qc in range(2):
                                nc.tensor.matmul(
                                    psg[:, qc * 512 : (qc + 1) * 512],
                                    w18[cp][:, :, r * 128 : (r + 1) * 128],
                                    h2T8[:, 2 * (cp % 3) : 2 * (cp % 3) + 2, qc * 512 : (qc + 1) * 512],
                                    start=(cp == 0),
                                    stop=(cp == 5),
                                    perf_mode=DR,
                                )
                        nc.scalar.activation(
                            gr[r // 2][:, r % 2, :],
                            psg[:],
                            AF.Gelu,
                            bias=bfc1_sb[:, r : r + 1],
                            scale=1.0 / WS,
                        )

            # ---------------- phase H: flipped fc2 (token-major out,
            # stationary = gr slice shared by hi+lo weights) + residual + store
            with (
                tc.tile_pool(name="w2p", bufs=24) as w2p,
                tc.tile_pool(name="obp", bufs=3) as obp,
            ):
                w28 = []
                for rp in range(24):  # 12 contraction pairs x {hi, lo}
                    wt = w2p.tile([128, 2, C], f8, tag="w2", name=f"w2_{rp}")
                    nc.sync.dma_start(wt[:], wfc2_d[rp])
                    w28.append(wt)
                with tc.tile_pool(name="psO", bufs=1, space="PSUM") as psO:
                    for t in range(NT):
                        pca = psO.tile([128, 512], f32, tag="pca", bufs=2, name="pca")
                        pcb = psO.tile([128, 256], f32, tag="pcb", bufs=2, name="pcb")
                        for rp in range(12):
                            grs = gr[rp][:, :, t * 128 : (t + 1) * 128]
                            last = rp == 11 and not has_fbias
                            nc.tensor.matmul(
                                pca[:], grs, w28[rp][:, :, 0:512],
                                start=(rp == 0), stop=False, perf_mode=DR,
                            )
                            nc.tensor.matmul(
                                pcb[:], grs, w28[rp][:, :, 512:C],
                                start=(rp == 0), stop=False, perf_mode=DR,
                            )
                            nc.tensor.matmul(
                                pca[:], grs, w28[12 + rp][:, :, 0:512],
                                start=False, stop=last, perf_mode=DR,
                            )
                            nc.tensor.matmul(
                                pcb[:], grs, w28[12 + rp][:, :, 512:C],
                                start=False, stop=last, perf_mode=DR,
                            )
                        if has_fbias:
                            nc.tensor.matmul(
                                pca[:], ones_bf[:], bf_row[:, 0:512],
                                start=False, stop=True,
                            )
                            nc.tensor.matmul(
                                pcb[:], ones_bf[:], bf_row[:, 512:C],
                                start=False, stop=True,
                            )
                        nc.vector.tensor_add(
                            xs[t][:, 0:512], xs[t][:, 0:512], pca[:]
                        )
                        nc.vector.tensor_add(
                            xs[t][:, 512:C], xs[t][:, 512:C], pcb[:]
                        )
                        ob = obp.tile([128, C], f32, tag="ob", name="ob")
                        nc.vector.tensor_scalar_mul(ob[:], xs[t][:], 1.0 / RS)
                        nc.sync.dma_start(out_d[t * 128 : (t + 1) * 128, :], ob[:])

    ndel = _dedupe_ldweights(nc)
    if os.environ.get("LDW_DEBUG"):
        print(f"deduped {ndel} ldweights")
    nc.compile()
    return nc


def _get_nc(reps=1, flags=(False, False, False)):
    key = f"nc{reps}_{flags}"
    if key not in _NC_CACHE:
        _NC_CACHE[key] = _build_nc(reps, *flags)
    return _NC_CACHE[key]


def _host_prep(inputs):
    import ml_dtypes

    f8 = ml_dtypes.float8_e4m3
    bf = ml_dtypes.bfloat16

    inp = {k: np.asarray(v) for k, v in inputs.items()}
    # residual stream carries RS; layernorm is scale-invariant
    x = np.ascontiguousarray(inp["x"] * RS, dtype=np.float32)  # [8, 1024, 768]
    g1 = inp["ln1_g"].astype(np.float64)
    b1 = inp["ln1_b"].astype(np.float64)
    qkv_w = inp["qkv_w"].astype(np.float64)  # [2304, 768]
    Ws = qkv_w.copy()
    Ws[:C] *= D ** (-0.5)  # fold attention scale into Wq
    WqT = np.ascontiguousarray((Ws * g1[None, :]).T)  # [768, 2304]
    wqkv8 = np.ascontiguousarray(
        (WqT * WS).reshape(3, 2, 128, 3 * C).transpose(0, 2, 1, 3)
    ).astype(f8)
    bqkv = (Ws @ b1).astype(np.float32).reshape(1, 3 * C)
    has_vbias = bool(np.any(bqkv[0, 2 * C :] != 0))
    bv = (bqkv[0, 2 * C :] * WS).astype(bf).reshape(1, C)

    projT = inp["proj_w"].astype(np.float64).T  # [768, 768]
    wproj8 = np.ascontiguousarray(
        (projT * WS).reshape(3, 2, 128, C).transpose(0, 2, 1, 3)
    ).astype(f8)
    bproj = inp["proj_b"].astype(np.float64)
    has_pbias = bool(np.any(bproj != 0))
    bprow = (bproj * RS).astype(bf).reshape(1, C)

    g2 = inp["ln2_g"].astype(np.float64)
    b2 = inp["ln2_b"].astype(np.float64)
    fc1_w = inp["fc1_w"].astype(np.float64)  # [3072, 768]

    def split_hi_lo(w_scaled, nparts, width):
        """[K, M] scaled weights -> [2*nparts, 128, 2, M] fp8 hi then lo."""
        tiles = np.ascontiguousarray(
            w_scaled.reshape(nparts, 2, 128, width).transpose(0, 2, 1, 3)
        )
        hi = tiles.astype(f8)
        lo = (tiles - hi.astype(np.float64)).astype(f8)
        return np.concatenate([hi, lo], axis=0)

    w1T = (fc1_w * g2[None, :]).T  # [768, 3072]
    wfc18 = split_hi_lo(w1T * WS, 3, F1)
    bfc1 = (fc1_w @ b2 + inp["fc1_b"].astype(np.float64)).astype(np.float32)
    bfc1 = bfc1.reshape(1, F1)
    w2T = inp["fc2_w"].astype(np.float64).T  # [3072, 768]
    wfc28 = split_hi_lo(w2T * RS, 12, C)
    bfc2 = inp["fc2_b"].astype(np.float64)
    has_fbias = bool(np.any(bfc2 != 0))
    bfrow = (bfc2 * RS).astype(bf).reshape(1, C)

    tab = inp["rel_table"].astype(np.float64)  # [129, 12]
    p_i = np.arange(128)[:, None]
    w_i = np.arange(256)[None, :]
    idx = np.clip(p_i - w_i + 127, 0, 128)
    patch = np.ascontiguousarray(
        (tab[idx, :] - tab[0, :]).transpose(2, 0, 1)
    ).astype(bf)  # [12, 128, 256]
    c2 = (tab[128, :] - tab[0, :]).astype(np.float32)  # [12]
    c2col = np.repeat(c2[:, None], 128, axis=1).reshape(1, H * 128).astype(bf)

    selm = np.zeros((2, 128), np.float32)
    selm[0, 0:64] = 8.0
    selm[1, 64:128] = 8.0
    selm = selm.astype(bf)

    shared = {
        "sel": selm,
        "wqkv8": wqkv8,
        "bqkv": bqkv,
        "bvrow": bv,
        "wproj8": wproj8,
        "bprow": bprow,
        "wfc18": wfc18,
        "bfc1": bfc1,
        "wfc28": wfc28,
        "bfrow": bfrow,
        "patch": patch,
        "c2col": c2col,
    }
    in_maps = [{"x": np.ascontiguousarray(x[c]), **shared} for c in range(B)]
    return in_maps, (has_vbias, has_pbias, has_fbias)


def _make_runner(reps=1, flags=(False, False, False)):
    import jax
    from jax.experimental.shard_map import shard_map
    from jax.sharding import Mesh, NamedSharding, PartitionSpec

    from concourse import bass2jax, mybir

    nc = _get_nc(reps, flags)
    bass2jax.install_neuronx_cc_hook()

    partition_name = nc.partition_id_tensor.name if nc.partition_id_tensor else None
    in_names, out_names, out_avals, zero_outs = [], [], [], []
    for alloc in nc.m.functions[0].allocations:
        if not isinstance(alloc, mybir.MemoryLocationSet):
            continue
        name = alloc.memorylocations[0].name
        if alloc.kind == "ExternalInput":
            if name != partition_name:
                in_names.append(name)
        elif alloc.kind == "ExternalOutput":
            out_names.append(name)
            shape = tuple(alloc.tensor_shape)
            dtype = mybir.dt.np(alloc.dtype)
            out_avals.append(jax.core.ShapedArray(shape, dtype))
            zero_outs.append(np.zeros(shape, dtype))
    n_params = len(in_names)
    all_names = tuple(in_names) + tuple(out_names)
    if partition_name is not None:
        all_names = all_names + (partition_name,)
    donate = tuple(range(n_params, n_params + len(out_names)))

    def _body(*args):
        operands = list(args)
        if partition_name is not None:
            operands.append(bass2jax.partition_id_tensor())
        outs = bass2jax._bass_exec_p.bind(
            *operands,
            out_avals=tuple(out_avals),
            in_names=all_names,
            out_names=tuple(out_names),
            lowering_input_output_aliases=(),
            sim_require_finite=True,
            sim_require_nnan=True,
            nc=nc,
        )
        return tuple(outs)

    def _body_k(k):
        def body(*args):
            ins = list(args[:n_params])
            outs = list(args[n_params:])
            for _ in range(k):
                outs = list(_body(*ins, *outs))
            return tuple(outs)

        return body

    devices = jax.devices()[:B]
    mesh = Mesh(np.asarray(devices), ("core",))
    in_specs = (PartitionSpec("core"),) * (n_params + len(out_names))
    out_specs = (PartitionSpec("core"),) * len(out_names)

    def make_fn(k):
        return jax.jit(
            shard_map(
                _body_k(k),
                mesh=mesh,
                in_specs=in_specs,
                out_specs=out_specs,
                check_rep=False,
            ),
            donate_argnums=donate,
            keep_unused=True,
        )

    sharding = NamedSharding(mesh, PartitionSpec("core"))
    return make_fn, in_names, out_names, zero_outs, sharding


def _get_runner(reps=1, flags=(False, False, False)):
    key = f"runner{reps}_{flags}"
    if key not in _NC_CACHE:
        _NC_CACHE[key] = _make_runner(reps, flags)
    return _NC_CACHE[key]


LAST_BENCH = None


def kernel(**inputs):
    global LAST_BENCH
    import time

    import jax

    in_maps, flags = _host_prep(inputs)
    make_fn, in_names, out_names, zero_outs, sharding = _get_runner(1, flags)
    concat_in = [
        np.concatenate([np.asarray(in_maps[c][n]) for c in range(B)], axis=0)
        for n in in_names
    ]
    concat_zeros = [
        np.zeros((B * z.shape[0], *z.shape[1:]), z.dtype) for z in zero_outs
    ]
    fn1 = make_fn(1)
    dev_in = [jax.device_put(a, sharding) for a in concat_in]
    outs = fn1(*dev_in, *concat_zeros)
    jax.block_until_ready(outs)
    result = np.asarray(outs[0]).reshape(B, N, C).astype(np.float32)

    iters = int(os.environ.get("BENCH_ITERS", "0"))
    if iters > 0:
        o = fn1(*dev_in, *outs)  # warm
        jax.block_until_ready(o)
        times = []
        for _ in range(iters):
            t0 = time.perf_counter()
            o = fn1(*dev_in, *o)
            jax.block_until_ready(o)
            times.append(time.perf_counter() - t0)
        overhead = _bench_overhead()
        t_min = float(np.min(times))
        t_med = float(np.median(times))
        LAST_BENCH = {
            "per_iter_ns": max(t_min - overhead, 0.0) * 1e9,
            "call_min_ns": t_min * 1e9,
            "call_med_ns": t_med * 1e9,
            "overhead_ns": overhead * 1e9,
            "iters": iters,
        }
    return result


def _bench_overhead():
    """Per-call dispatch overhead, measured with a trivial 1-DMA kernel."""
    import time

    import jax
    from jax.experimental.shard_map import shard_map
    from jax.sharding import Mesh, PartitionSpec

    import concourse.bacc as bacc
    import concourse.tile as tile
    from concourse import bass2jax, mybir

    if "tiny" not in _NC_CACHE:
        f32 = mybir.dt.float32
        nc = bacc.Bacc(
            "TRN2",
            target_bir_lowering=False,
            debug=False,
            enable_asserts=False,
            num_devices=8,
        )
        xi = nc.dram_tensor("ti", [128, 128], f32, kind="ExternalInput").ap()
        xo = nc.dram_tensor("to", [128, 128], f32, kind="ExternalOutput").ap()
        with tile.TileContext(nc) as tc:
            with tc.tile_pool(name="p", bufs=1) as p:
                t = p.tile([128, 128], f32, tag="t", name="t")
                nc.sync.dma_start(t[:], xi[:])
                nc.sync.dma_start(xo[:], t[:])
        nc.compile()

        partition_name = nc.partition_id_tensor.name if nc.partition_id_tensor else None
        all_names = ["ti", "to"]
        if partition_name is not None:
            all_names.append(partition_name)
        out_avals = [jax.core.ShapedArray((128, 128), np.float32)]

        def _tbody(*args):
            operands = list(args)
            if partition_name is not None:
                operands.append(bass2jax.partition_id_tensor())
            return tuple(
                bass2jax._bass_exec_p.bind(
                    *operands,
                    out_avals=tuple(out_avals),
                    in_names=tuple(all_names),
                    out_names=("to",),
                    lowering_input_output_aliases=(),
                    sim_require_finite=True,
                    sim_require_nnan=True,
                    nc=nc,
                )
            )

        devices = jax.devices()[:B]
        mesh = Mesh(np.asarray(devices), ("core",))
        tfn = jax.jit(
            shard_map(
                _tbody,
                mesh=mesh,
                in_specs=(PartitionSpec("core"),) * 2,
                out_specs=(PartitionSpec("core"),),
                check_rep=False,
            ),
            donate_argnums=(1,),
            keep_unused=True,
        )
        _NC_CACHE["tiny"] = tfn

    tfn = _NC_CACHE["tiny"]
    ti = np.zeros((B * 128, 128), np.float32)
    o = tfn(ti, np.zeros((B * 128, 128), np.float32))
    jax.block_until_ready(o)
    times = []
    for _ in range(30):
        t0 = time.perf_counter()
        o = tfn(ti, *([o] if not isinstance(o, tuple) else list(o)))
        jax.block_until_ready(o)
        times.append(time.perf_counter() - t0)
    return float(np.min(times))


# revision 34
# speedup vs baseline: 1.0451x; 1.0359x over previous
"""Trainium2 Bass kernel for a dense transformer block (B=8, N=1024, C=768, H=12).

Sharding: data-parallel over batch -- one batch element per NeuronCore (8 cores),
weights replicated, no collectives.

v3: fp8e4 DoubleRow matmuls (256-deep contraction @ 0.5 cyc/row) for
QKV/V/proj/FC1/FC2/AV, bf16 row-packed score matmuls, bf16 transposes.

Rel-bias via softmax shift-invariance: subtract table[0]; below-diagonal
regions need nothing, the above-diagonal constant c2=table[128]-table[0] is a
rank-1 PE accumulate, and the 255-wide diagonal window is a PE identity-matmul
accumulate from a host-built [128,256] Toeplitz patch.

Residual stream carries a global x512 scale (x is pre-scaled on host;
layernorm is scale-invariant) so that proj (8*64) and fc2 (1*512) PSUMs add
straight into it; one tensor_scalar * (1/512) at the output.

proj and fc2 are "flipped" (stationary = activations, moving = weights, output
token-major) so one LDWEIGHTS serves several matmuls and no output transposes
are needed. fc1/fc2 weights ship as fp8 hi+lo pairs (lo rides in e4m3
subnormals) for a 2-term residual quantization. A post-schedule pass dedupes
back-to-back identical LDWEIGHTS, which the stack otherwise emits 1:1 per
matmul.
"""

import os

import numpy as np

B, N, C, H, D = 8, 1024, 768, 12, 64
NT = N // 128   # 8 token tiles
KT = C // 128   # 6 feature tiles
F1 = 4 * C      # 3072
RT = F1 // 128  # 24
EPS = 1e-5
WS = 64.0       # fp8 weight scale (qkv, proj, fc1)
RS = 512.0      # residual-stream scale; fc2 weight scale

LAST_RESULTS = None

_NC_CACHE = {}


def _dedupe_ldweights(nc):
    """Drop InstLdweights identical to the immediately-preceding one.

    The scheduler emits one Ldweights per matmul even when consecutive
    matmuls share the stationary operand; the duplicate loads are pure
    weight-port waste (256 cols @ 1.2 GHz each for DoubleRow)."""
    ndel = 0
    remap = {}
    for fn in nc.m.functions:
        for blk in fn.blocks:
            insts = list(blk.instructions)
            keep = []
            last_sig = None
            last_name = None
            changed = False
            for inst in insts:
                tn = type(inst).__name__
                if tn == "InstLdweights":
                    c = inst.concise()
                    sig = (
                        c.split("in=")[-1],
                        str(inst.perf_mode),
                        str(inst.is_transpose),
                        str(inst.tile_position),
                        tuple(sorted(inst.sync_dependency_names())),
                    )
                    if sig == last_sig and "wait:" not in c:
                        remap[inst.name] = last_name
                        ndel += 1
                        changed = True
                        continue
                    last_sig = sig
                    last_name = inst.name
                elif tn == "InstMatmult":
                    if inst.ldweights:
                        last_sig = None
                keep.append(inst)
            if changed:
                blk.instructions = keep
    if remap:
        for fn in nc.m.functions:
            for blk in fn.blocks:
                for inst in blk.instructions:
                    deps = set(inst.sync_dependency_names()) | set(
                        inst.nosync_dependency_names()
                    )
                    hits = deps & set(remap)
                    if hits:
                        inst.remap_dependency_names(
                            {old: remap[old] for old in hits}
                        )
    return ndel


def _build_nc(reps=1, has_vbias=False, has_pbias=False, has_fbias=False):
    from contextlib import ExitStack

    import concourse.bacc as bacc
    import concourse.tile as tile
    from concourse import masks, mybir

    f32 = mybir.dt.float32
    bf16 = mybir.dt.bfloat16
    f8 = mybir.dt.float8e4
    DR = mybir.MatmulPerfMode.DoubleRow

    AF = mybir.ActivationFunctionType
    AX = mybir.AxisListType
    OP = mybir.AluOpType

    nc = bacc.Bacc(
        "TRN2",
        target_bir_lowering=False,
        debug=False,
        enable_asserts=False,
        num_devices=8,
    )

    x_d = nc.dram_tensor("x", [N, C], f32, kind="ExternalInput").ap()
    wqkv_d = nc.dram_tensor("wqkv8", [3, 128, 2, 3 * C], f8, kind="ExternalInput").ap()
    bqkv_d = nc.dram_tensor("bqkv", [1, 3 * C], f32, kind="ExternalInput").ap()
    bv_d = nc.dram_tensor("bvrow", [1, C], bf16, kind="ExternalInput").ap()
    wproj_d = nc.dram_tensor("wproj8", [3, 128, 2, C], f8, kind="ExternalInput").ap()
    bproj_d = nc.dram_tensor("bprow", [1, C], bf16, kind="ExternalInput").ap()
    wfc1_d = nc.dram_tensor("wfc18", [6, 128, 2, F1], f8, kind="ExternalInput").ap()
    bfc1_d = nc.dram_tensor("bfc1", [1, F1], f32, kind="ExternalInput").ap()
    wfc2_d = nc.dram_tensor("wfc28", [24, 128, 2, C], f8, kind="ExternalInput").ap()
    bfc2_d = nc.dram_tensor("bfrow", [1, C], bf16, kind="ExternalInput").ap()
    patch_d = nc.dram_tensor("patch", [H, 128, 256], bf16, kind="ExternalInput").ap()
    c2_d = nc.dram_tensor("c2col", [1, H * 128], bf16, kind="ExternalInput").ap()
    sel_d = nc.dram_tensor("sel", [2, 128], bf16, kind="ExternalInput").ap()
    out_d = nc.dram_tensor("out", [N, C], f32, kind="ExternalOutput").ap()

    with tile.TileContext(nc) as tc, ExitStack() as ctx:
        cpool = ctx.enter_context(tc.tile_pool(name="const", bufs=1))
        identb = cpool.tile([128, 128], bf16, tag="identb")
        masks.make_identity(nc, identb[:])
        ones_bf = cpool.tile([1, 128], bf16, tag="onesb")
        nc.any.memset(ones_bf[:], 1.0)
        ones_row = cpool.tile([1, N], bf16, tag="onesrow")
        nc.any.memset(ones_row[:], 1.0)
        sel = cpool.tile([2, 128], bf16, tag="sel")
        nc.sync.dma_start(sel[:], sel_d[:])
        epsc = cpool.tile([128, 1], f32, tag="eps")
        nc.any.memset(epsc[:], EPS * RS * RS)
        patch = cpool.tile([128, H, 256], bf16, tag="patch")
        for h in range(H):
            nc.sync.dma_start(patch[:, h, :], patch_d[h])
        c2col = cpool.tile([1, H * 128], bf16, tag="c2col")
        nc.sync.dma_start(c2col[:], c2_d[:])
        bqkv_sb = cpool.tile([128, 18], f32, tag="bqkv")
        nc.sync.dma_start(bqkv_sb[:], bqkv_d[0].rearrange("(a p) -> p a", p=128))
        bv_row = cpool.tile([1, C], bf16, tag="bvrow")
        nc.sync.dma_start(bv_row[:], bv_d[:])
        bp_row = cpool.tile([1, C], bf16, tag="bprow")
        nc.sync.dma_start(bp_row[:], bproj_d[:])
        bfc1_sb = cpool.tile([128, RT], f32, tag="bfc1")
        nc.sync.dma_start(bfc1_sb[:], bfc1_d[0].rearrange("(a p) -> p a", p=128))
        bf_row = cpool.tile([1, C], bf16, tag="bfrow")
        nc.sync.dma_start(bf_row[:], bfc2_d[:])

        stat = ctx.enter_context(tc.tile_pool(name="stat", bufs=8))
        chain = ctx.enter_context(tc.tile_pool(name="chain", bufs=1))

        def layernorm(dst_ap, src_ap, scratch_ap):
            """dst = (src - mean(src)) * rsqrt(var(src) + eps').

            src carries the RS scale; eps' = EPS*RS^2 keeps the result equal
            to layernorm of src/RS. The square runs on GpSimd (SBUF-only)."""
            sums = stat.tile([128, 1], f32, tag="sums", name="sums")
            nc.vector.reduce_sum(sums[:], src_ap, axis=AX.X)
            mu = stat.tile([128, 1], f32, tag="mu", name="mu")
            nc.vector.tensor_scalar_mul(mu[:], sums[:], 1.0 / C)
            nc.gpsimd.tensor_mul(scratch_ap, src_ap, src_ap)
            ssq = stat.tile([128, 1], f32, tag="ssq", name="ssq")
            nc.vector.reduce_sum(ssq[:], scratch_ap, axis=AX.X)
            musq = stat.tile([128, 1], f32, tag="musq", name="musq")
            nc.vector.tensor_mul(musq[:], mu[:], mu[:])
            var = stat.tile([128, 1], f32, tag="var", name="var")
            nc.vector.tensor_scalar(
                var[:], ssq[:], 1.0 / C, musq[:], op0=OP.mult, op1=OP.subtract
            )
            sd = stat.tile([128, 1], f32, tag="sd", name="sd")
            nc.scalar.activation(sd[:], var[:], AF.Sqrt, bias=epsc[:])
            rstd = stat.tile([128, 1], f32, tag="rstd", name="rstd")
            nc.vector.reciprocal(rstd[:], sd[:])
            nmr = stat.tile([128, 1], f32, tag="nmr", name="nmr")
            nc.vector.tensor_scalar(
                nmr[:], mu[:], rstd[:], -1.0, op0=OP.mult, op1=OP.mult
            )
            nc.vector.tensor_scalar(
                dst_ap, src_ap, rstd[:], nmr[:], op0=OP.mult, op1=OP.add
            )

        for _rep in range(reps):
            xs = [chain.tile([128, C], f32, tag="x", bufs=NT, name=f"x{t}") for t in range(NT)]
            hT8 = chain.tile([128, KT, N], f8, tag="hT8", bufs=1, name="hT8")
            qkT = [
                chain.tile([128, N], bf16, tag="qkT", bufs=12, name=f"qkT{i}")
                for i in range(12)
            ]
            vaug = chain.tile([128, NT, H, 72], f8, tag="vaug", bufs=1, name="vaug")
            aT8 = chain.tile([128, KT, N], f8, tag="aT8", bufs=1, name="aT8")
            s_all = chain.tile([H, N], f32, tag="sall", bufs=1, name="sall")
            s8 = chain.tile([H, N], bf16, tag="s8", bufs=1, name="s8")

            # ---------------- phase A: load x (pre-scaled by RS on host),
            # LN1, transpose -> hT8 (fp8)
            with tc.tile_pool(name="psA", bufs=6, space="PSUM") as psA:
                for t in range(NT):
                    nc.sync.dma_start(xs[t][:], x_d[t * 128 : (t + 1) * 128, :])
                    h1 = chain.tile([128, C], bf16, tag="hln", bufs=3, name=f"h1_{t}")
                    scr = chain.tile([128, C], f32, tag="hscr", bufs=3, name=f"sc1_{t}")
                    layernorm(h1[:], xs[t][:], scr[:])
                    for ct in range(KT):
                        ps = psA.tile([128, 128], bf16, tag="tp", name="psa")
                        nc.tensor.transpose(
                            ps[:], h1[:, ct * 128 : (ct + 1) * 128], identb[:]
                        )
                        nc.vector.tensor_copy(
                            hT8[:, ct, t * 128 : (t + 1) * 128], ps[:]
                        )

            # ---------------- phase C: QKV via fp8 DoubleRow
            with tc.tile_pool(name="wqkv", bufs=3) as wq_pool:
                wq8 = []
                for cp in range(3):
                    wt = wq_pool.tile([128, 2, 3 * C], f8, tag="wq", name=f"wq{cp}")
                    nc.sync.dma_start(wt[:], wqkv_d[cp])
                    wq8.append(wt)
                with tc.tile_pool(name="psC", bufs=1, space="PSUM") as psC:
                    for jt in range(12):
                        pss = [
                            psC.tile([128, 512], f32, tag="ps", bufs=4, name=f"psc{qc}")
                            for qc in range(2)
                        ]
                        for cp in range(3):
                            for qc in range(2):
                                nc.tensor.matmul(
                                    pss[qc][:],
                                    wq8[cp][:, :, jt * 128 : (jt + 1) * 128],
                                    hT8[:, 2 * cp : 2 * cp + 2, qc * 512 : (qc + 1) * 512],
                                    start=(cp == 0),
                                    stop=(cp == 2),
                                    perf_mode=DR,
                                )
                        for qc in range(2):
                            nc.vector.tensor_scalar(
                                qkT[jt][:, qc * 512 : (qc + 1) * 512],
                                pss[qc][:],
                                1.0 / WS,
                                bqkv_sb[:, jt : jt + 1],
                                op0=OP.mult,
                                op1=OP.add,
                            )
                    # v token-major: lhsT = hT8 (stationary), rhs = wv
                    for t in range(NT):
                        psa = psC.tile([128, 512], f32, tag="psva", bufs=2, name="psva")
                        psb2 = psC.tile([128, 256], f32, tag="psvb", bufs=2, name="psvb")
                        for cp in range(3):
                            last = cp == 2 and not has_vbias
                            nc.tensor.matmul(
                                psa[:],
                                hT8[:, 2 * cp : 2 * cp + 2, t * 128 : (t + 1) * 128],
                                wq8[cp][:, :, 2 * C : 2 * C + 512],
                                start=(cp == 0),
                                stop=last,
                                perf_mode=DR,
                            )
                            nc.tensor.matmul(
                                psb2[:],
                                hT8[:, 2 * cp : 2 * cp + 2, t * 128 : (t + 1) * 128],
                                wq8[cp][:, :, 2 * C + 512 : 3 * C],
                                start=(cp == 0),
                                stop=last,
                                perf_mode=DR,
                            )
                        if has_vbias:
                            nc.tensor.matmul(
                                psa[:], ones_bf[:], bv_row[:, 0:512],
                                start=False, stop=True,
                            )
                            nc.tensor.matmul(
                                psb2[:], ones_bf[:], bv_row[:, 512:C],
                                start=False, stop=True,
                            )
                        nc.vector.tensor_scalar_mul(
                            vaug[:, t, 0:8, 0:64],
                            psa[:].rearrange("p (h e) -> p h e", e=64),
                            1.0 / WS,
                        )
                        nc.vector.tensor_scalar_mul(
                            vaug[:, t, 8:12, 0:64],
                            psb2[:].rearrange("p (h e) -> p h e", e=64),
                            1.0 / WS,
                        )
                nc.any.memset(vaug[:, :, :, 64:65], 1.0)
                nc.any.memset(vaug[:, :, :, 65:72], 0.0)

            # ---------------- phase D: attention scores + exp + AV
            with (
                tc.tile_pool(name="ptp", bufs=3) as pt_pool,
                tc.tile_pool(name="psS", bufs=2, space="PSUM") as psS,
                tc.tile_pool(name="psAV", bufs=4, space="PSUM") as psAV,
                tc.tile_pool(name="oddp", bufs=3) as oddp,
            ):
                for hp in range(KT):
                    pts = [
                        pt_pool.tile([128, NT, N], f8, tag="pt", name=f"pt{hp}_{o}")
                        for o in range(2)
                    ]
                    for kc in range(NT):
                        jA = max(0, kc * 128 - 63)
                        jB = min(N, kc * 128 + 192)
                        w0 = 63 if kc == 0 else 0
                        c2chunks = []
                        off = 0
                        while off < jA:
                            w = min(512, jA - off)
                            c2chunks.append((off, w))
                            off += w
                        # even/odd score matmuls adjacent: they target disjoint
                        # PE row groups (base partition 0 vs 64) and overlap
                        pse_o = [
                            psS.tile([128, N], f32, tag="ps", bufs=3, name=f"pss{o}")
                            for o in range(2)
                        ]
                        for odd in range(2):
                            ro = odd * 64
                            for qc in range(2):
                                nc.tensor.matmul(
                                    pse_o[odd][:, qc * 512 : (qc + 1) * 512],
                                    qkT[6 + hp][ro : ro + 64, kc * 128 : (kc + 1) * 128],
                                    qkT[hp][ro : ro + 64, qc * 512 : (qc + 1) * 512],
                                    start=True,
                                    stop=True,
                                )
                        for odd in range(2):
                            h = 2 * hp + odd
                            ps = pse_o[odd]
                            for off, w in c2chunks:
                                nc.tensor.matmul(
                                    ps[:, off : off + w],
                                    c2col[:, h * 128 : (h + 1) * 128],
                                    ones_row[:, 0:w],
                                    start=False,
                                    stop=True,
                                    skip_group_check=True,
                                )
                            # diagonal-window rel-bias: accumulate the Toeplitz
                            # patch via identity matmul (split at PSUM banks)
                            segs = [
                                (a, b)
                                for a, b in ((jA, min(jB, 512)), (max(jA, 512), jB))
                                if b > a
                            ]
                            for a, b in segs:
                                nc.tensor.matmul(
                                    ps[:, a:b],
                                    identb[:],
                                    patch[:, h, w0 + (a - jA) : w0 + (b - jA)],
                                    start=False,
                                    stop=True,
                                    skip_group_check=True,
                                )
                            nc.scalar.activation(pts[odd][:, kc, :], ps[:], AF.Exp)
                    # AV with DoubleRow over kc pairs; ones col gives sums
                    for odd in range(2):
                        h = 2 * hp + odd
                        pavs = [
                            psAV.tile([72, 512], f32, tag="pav", bufs=2, name=f"pav{qc}")
                            for qc in range(2)
                        ]
                        for m in range(4):
                            for qc in range(2):
                                nc.tensor.matmul(
                                    pavs[qc][:],
                                    vaug[:, 2 * m : 2 * m + 2, h, :],
                                    pts[odd][:, 2 * m : 2 * m + 2, qc * 512 : (qc + 1) * 512],
                                    start=(m == 0),
                                    stop=(m == 3),
                                    perf_mode=DR,
                                )
                        for qc in range(2):
                            pav = pavs[qc]
                            srow = oddp.tile([128, 512], f32, tag="srow", name="srow")
                            nc.vector.tensor_copy(srow[64:65, :], pav[64:65, :])
                            nc.sync.dma_start(
                                s_all[h : h + 1, qc * 512 : (qc + 1) * 512],
                                srow[64:65, :],
                            )
                            if odd:
                                tmp = oddp.tile([64, 512], f8, tag="odd", name="avodd")
                                nc.vector.tensor_copy(tmp[:], pav[0:64, :])
                                nc.sync.dma_start(
                                    aT8[64:128, hp, qc * 512 : (qc + 1) * 512], tmp[:]
                                )
                            else:
                                nc.vector.tensor_copy(
                                    aT8[0:64, hp, qc * 512 : (qc + 1) * 512],
                                    pav[0:64, :],
                                )

            # ---------------- phase E: normalize aT8, then flipped proj
            # (token-major out) + residual into xs
            with tc.tile_pool(name="wpp", bufs=3) as wpp:
                wp8 = []
                for hp2 in range(3):
                    wt = wpp.tile([128, 2, C], f8, tag="wp", name=f"wp{hp2}")
                    nc.sync.dma_start(wt[:], wproj_d[hp2])
                    wp8.append(wt)
                with (
                    tc.tile_pool(name="st2p", bufs=4) as st2p,
                    tc.tile_pool(name="psNorm", bufs=2, space="PSUM") as psN,
                ):
                    nc.vector.reciprocal_approx_fast(s_all[:], s_all[:])
                    nc.vector.tensor_copy(s8[:], s_all[:])
                    for qc in range(2):
                        for hp in range(KT):
                            st2 = st2p.tile([2, 512], bf16, tag="stg", name="st2")
                            nc.sync.dma_start(
                                st2[:],
                                s8[2 * hp : 2 * hp + 2, qc * 512 : (qc + 1) * 512],
                            )
                            psb = psN.tile([128, 512], f32, tag="psn", name="psn")
                            nc.tensor.matmul(
                                psb[0:64, :], sel[:, 0:64], st2[:], start=True, stop=True
                            )
                            nc.tensor.matmul(
                                psb[64:128, :], sel[:, 64:128], st2[:],
                                start=True, stop=True,
                            )
                            nc.vector.tensor_mul(
                                aT8[:, hp, qc * 512 : (qc + 1) * 512],
                                aT8[:, hp, qc * 512 : (qc + 1) * 512],
                                psb[:],
                            )
                with tc.tile_pool(name="psP", bufs=1, space="PSUM") as psP:
                    for t in range(NT):
                        pja = psP.tile([128, 512], f32, tag="pja", bufs=2, name="pja")
                        pjb = psP.tile([128, 256], f32, tag="pjb", bufs=2, name="pjb")
                        for hp2 in range(3):
                            last = hp2 == 2 and not has_pbias
                            nc.tensor.matmul(
                                pja[:],
                                aT8[:, 2 * hp2 : 2 * hp2 + 2, t * 128 : (t + 1) * 128],
                                wp8[hp2][:, :, 0:512],
                                start=(hp2 == 0),
                                stop=last,
                                perf_mode=DR,
                            )
                            nc.tensor.matmul(
                                pjb[:],
                                aT8[:, 2 * hp2 : 2 * hp2 + 2, t * 128 : (t + 1) * 128],
                                wp8[hp2][:, :, 512:C],
                                start=(hp2 == 0),
                                stop=last,
                                perf_mode=DR,
                            )
                        if has_pbias:
                            nc.tensor.matmul(
                                pja[:], ones_bf[:], bp_row[:, 0:512],
                                start=False, stop=True,
                            )
                            nc.tensor.matmul(
                                pjb[:], ones_bf[:], bp_row[:, 512:C],
                                start=False, stop=True,
                            )
                        nc.vector.tensor_add(
                            xs[t][:, 0:512], xs[t][:, 0:512], pja[:]
                        )
                        nc.vector.tensor_add(
                            xs[t][:, 512:C], xs[t][:, 512:C], pjb[:]
                        )

            # ---------------- phase F: LN2 -> h2T8 (reuses hT8's buffer)
            h2T8 = chain.tile([128, KT, N], f8, tag="hT8", bufs=1, name="h2T8")
            with tc.tile_pool(name="psF", bufs=6, space="PSUM") as psF:
                for t in range(NT):
                    h2 = chain.tile([128, C], bf16, tag="hln", bufs=3, name=f"h2_{t}")
                    scr = chain.tile([128, C], f32, tag="hscr", bufs=3, name=f"sc2_{t}")
                    layernorm(h2[:], xs[t][:], scr[:])
                    for ct in range(KT):
                        ps = psF.tile([128, 128], bf16, tag="tp", name="psf")
                        nc.tensor.transpose(
                            ps[:], h2[:, ct * 128 : (ct + 1) * 128], identb[:]
                        )
                        nc.vector.tensor_copy(
                            h2T8[:, ct, t * 128 : (t + 1) * 128], ps[:]
                        )

            # ---------------- phase G: fc1 + gelu -> gr tiles
            # (reuse qkT's 12 x 2KB buffers: qkT is dead after phase D)
            gr = [
                chain.tile([128, 2, N], f8, tag="qkT", bufs=12, name=f"gr{rp}")
                for rp in range(12)
            ]
            with tc.tile_pool(name="w1p", bufs=6) as w1p:
                w18 = []
                for cp in range(6):  # 3 contraction pairs x {hi, lo}
                    wt = w1p.tile([128, 2, F1], f8, tag="w1", name=f"w1_{cp}")
                    nc.sync.dma_start(wt[:], wfc1_d[cp])
                    w18.append(wt)
                with tc.tile_pool(name="psG", bufs=2, space="PSUM") as psG:
                    for r in range(RT):
                        psg = psG.tile([128, N], f32, tag="psg", name="psg")
                        for cp in range(6):
                            for qc in range(2):
                                nc.tensor.matmul(
                                    psg[:, qc * 512 : (qc + 1) * 512],
                                    w18[cp][:, :, r * 128 : (r + 1) * 128],
                                    h2T8[:, 2 * (cp % 3) : 2 * (cp % 3) + 2, qc * 512 : (qc + 1) * 512],
                                    start=(cp == 0),
                                    stop=(cp == 5),
                                    perf_mode=DR,
                                )
                        nc.scalar.activation(
                            gr[r // 2][:, r % 2, :],
                            psg[:],
                            AF.Gelu,
                            bias=bfc1_sb[:, r : r + 1],
                            scale=1.0 / WS,
                        )

            # ---------------- phase H: flipped fc2 (token-major out,
            # stationary = gr slice shared by hi+lo weights) + residual + store
            with (
                tc.tile_pool(name="w2p", bufs=24) as w2p,
                tc.tile_pool(name="obp", bufs=3) as obp,
            ):
                w28 = []
                for rp in range(24):  # 12 contraction pairs x {hi, lo}
                    wt = w2p.tile([128, 2, C], f8, tag="w2", name=f"w2_{rp}")
                    nc.sync.dma_start(wt[:], wfc2_d[rp])
                    w28.append(wt)
                with tc.tile_pool(name="psO", bufs=1, space="PSUM") as psO:
                    for t in range(NT):
                        pca = psO.tile([128, 512], f32, tag="pca", bufs=2, name="pca")
                        pcb = psO.tile([128, 256], f32, tag="pcb", bufs=2, name="pcb")
                        for rp in range(12):
                            grs = gr[rp][:, :, t * 128 : (t + 1) * 128]
                            last = rp == 11 and not has_fbias
                            nc.tensor.matmul(
                                pca[:], grs, w28[rp][:, :, 0:512],
                                start=(rp == 0), stop=False, perf_mode=DR,
                            )
                            nc.tensor.matmul(
                                pcb[:], grs, w28[rp][:, :, 512:C],
                                start=(rp == 0), stop=False, perf_mode=DR,
                            )
                            nc.tensor.matmul(
                                pca[:], grs, w28[12 + rp][:, :, 0:512],
                                start=False, stop=last, perf_mode=DR,
                            )
                            nc.tensor.matmul(
                                pcb[:], grs, w28[12 + rp][:, :, 512:C],
                                start=False, stop=last, perf_mode=DR,
                            )
                        if has_fbias:
                            nc.tensor.matmul(
                                pca[:], ones_bf[:], bf_row[:, 0:512],
                                start=False, stop=True,
                            )
                            nc.tensor.matmul(
                                pcb[:], ones_bf[:], bf_row[:, 512:C],
                                start=False, stop=True,
                            )
                        nc.vector.tensor_add(
                            xs[t][:, 0:512], xs[t][:, 0:512], pca[:]
                        )
                        nc.vector.tensor_add(
                            xs[t][:, 512:C], xs[t][:, 512:C], pcb[:]
                        )
                        ob = obp.tile([128, C], f32, tag="ob", name="ob")
                        nc.vector.tensor_scalar_mul(ob[:], xs[t][:], 1.0 / RS)
                        nc.sync.dma_start(out_d[t * 128 : (t + 1) * 128, :], ob[:])

    ndel = _dedupe_ldweights(nc)
    if os.environ.get("LDW_DEBUG"):
        print(f"deduped {ndel} ldweights")
    nc.compile()
    return nc


def _get_nc(reps=1, flags=(False, False, False)):
    key = f"nc{reps}_{flags}"
    if key not in _NC_CACHE:
        _NC_CACHE[key] = _build_nc(reps, *flags)
    return _NC_CACHE[key]


def _host_prep(inputs):
    import ml_dtypes

    f8 = ml_dtypes.float8_e4m3
    bf = ml_dtypes.bfloat16

    inp = {k: np.asarray(v) for k, v in inputs.items()}
    # residual stream carries RS; layernorm is scale-invariant
    x = np.ascontiguousarray(inp["x"] * RS, dtype=np.float32)  # [8, 1024, 768]
    g1 = inp["ln1_g"].astype(np.float64)
    b1 = inp["ln1_b"].astype(np.float64)
    qkv_w = inp["qkv_w"].astype(np.float64)  # [2304, 768]
    Ws = qkv_w.copy()
    Ws[:C] *= D ** (-0.5)  # fold attention scale into Wq
    WqT = np.ascontiguousarray((Ws * g1[None, :]).T)  # [768, 2304]
    wqkv8 = np.ascontiguousarray(
        (WqT * WS).reshape(3, 2, 128, 3 * C).transpose(0, 2, 1, 3)
    ).astype(f8)
    bqkv = (Ws @ b1).astype(np.float32).reshape(1, 3 * C)
    has_vbias = bool(np.any(bqkv[0, 2 * C :] != 0))
    bv = (bqkv[0, 2 * C :] * WS).astype(bf).reshape(1, C)

    projT = inp["proj_w"].astype(np.float64).T  # [768, 768]
    wproj8 = np.ascontiguousarray(
        (projT * WS).reshape(3, 2, 128, C).transpose(0, 2, 1, 3)
    ).astype(f8)
    bproj = inp["proj_b"].astype(np.float64)
    has_pbias = bool(np.any(bproj != 0))
    bprow = (bproj * RS).astype(bf).reshape(1, C)

    g2 = inp["ln2_g"].astype(np.float64)
    b2 = inp["ln2_b"].astype(np.float64)
    fc1_w = inp["fc1_w"].astype(np.float64)  # [3072, 768]

    def split_hi_lo(w_scaled, nparts, width):
        """[K, M] scaled weights -> [2*nparts, 128, 2, M] fp8 hi then lo."""
        tiles = np.ascontiguousarray(
            w_scaled.reshape(nparts, 2, 128, width).transpose(0, 2, 1, 3)
        )
        hi = tiles.astype(f8)
        lo = (tiles - hi.astype(np.float64)).astype(f8)
        return np.concatenate([hi, lo], axis=0)

    w1T = (fc1_w * g2[None, :]).T  # [768, 3072]
    wfc18 = split_hi_lo(w1T * WS, 3, F1)
    bfc1 = (fc1_w @ b2 + inp["fc1_b"].astype(np.float64)).astype(np.float32)
    bfc1 = bfc1.reshape(1, F1)
    w2T = inp["fc2_w"].astype(np.float64).T  # [3072, 768]
    wfc28 = split_hi_lo(w2T * RS, 12, C)
    bfc2 = inp["fc2_b"].astype(np.float64)
    has_fbias = bool(np.any(bfc2 != 0))
    bfrow = (bfc2 * RS).astype(bf).reshape(1, C)

    tab = inp["rel_table"].astype(np.float64)  # [129, 12]
    p_i = np.arange(128)[:, None]
    w_i = np.arange(256)[None, :]
    idx = np.clip(p_i - w_i + 127, 0, 128)
    patch = np.ascontiguousarray(
        (tab[idx, :] - tab[0, :]).transpose(2, 0, 1)
    ).astype(bf)  # [12, 128, 256]
    c2 = (tab[128, :] - tab[0, :]).astype(np.float32)  # [12]
    c2col = np.repeat(c2[:, None], 128, axis=1).reshape(1, H * 128).astype(bf)

    selm = np.zeros((2, 128), np.float32)
    selm[0, 0:64] = 8.0
    selm[1, 64:128] = 8.0
    selm = selm.astype(bf)

    shared = {
        "sel": selm,
        "wqkv8": wqkv8,
        "bqkv": bqkv,
        "bvrow": bv,
        "wproj8": wproj8,
        "bprow": bprow,
        "wfc18": wfc18,
        "bfc1": bfc1,
        "wfc28": wfc28,
        "bfrow": bfrow,
        "patch": patch,
        "c2col": c2col,
    }
    in_maps = [{"x": np.ascontiguousarray(x[c]), **shared} for c in range(B)]
    return in_maps, (has_vbias, has_pbias, has_fbias)


def _make_runner(reps=1, flags=(False, False, False)):
    import jax
    from jax.experimental.shard_map import shard_map
    from jax.sharding import Mesh, NamedSharding, PartitionSpec

    from concourse import bass2jax, mybir

    nc = _get_nc(reps, flags)
    bass2jax.install_neuronx_cc_hook()

    partition_name = nc.partition_id_tensor.name if nc.partition_id_tensor else None
    in_names, out_names, out_avals, zero_outs = [], [], [], []
    for alloc in nc.m.functions[0].allocations:
        if not isinstance(alloc, mybir.MemoryLocationSet):
            continue
        name = alloc.memorylocations[0].name
        if alloc.kind == "ExternalInput":
            if name != partition_name:
                in_names.append(name)
        elif alloc.kind == "ExternalOutput":
            out_names.append(name)
            shape = tuple(alloc.tensor_shape)
            dtype = mybir.dt.np(alloc.dtype)
            out_avals.append(jax.core.ShapedArray(shape, dtype))
            zero_outs.append(np.zeros(shape, dtype))
    n_params = len(in_names)
    all_names = tuple(in_names) + tuple(out_names)
    if partition_name is not None:
        all_names = all_names + (partition_name,)
    donate = tuple(range(n_params, n_params + len(out_names)))

    def _body(*args):
        operands = list(args)
        if partition_name is not None:
            operands.append(bass2jax.partition_id_tensor())
        outs = bass2jax._bass_exec_p.bind(
            *operands,
            out_avals=tuple(out_avals),
            in_names=all_names,
            out_names=tuple(out_names),
            lowering_input_output_aliases=(),
            sim_require_finite=True,
            sim_require_nnan=True,
            nc=nc,
        )
        return tuple(outs)

    def _body_k(k):
        def body(*args):
            ins = list(args[:n_params])
            outs = list(args[n_params:])
            for _ in range(k):
                outs = list(_body(*ins, *outs))
            return tuple(outs)

        return body

    devices = jax.devices()[:B]
    mesh = Mesh(np.asarray(devices), ("core",))
    in_specs = (PartitionSpec("core"),) * (n_params + len(out_names))
    out_specs = (PartitionSpec("core"),) * len(out_names)

    def make_fn(k):
        return jax.jit(
            shard_map(
                _body_k(k),
                mesh=mesh,
                in_specs=in_specs,
                out_specs=out_specs,
                check_rep=False,
            ),
            donate_argnums=donate,
            keep_unused=True,
        )

    sharding = NamedSharding(mesh, PartitionSpec("core"))
    return make_fn, in_names, out_names, zero_outs, sharding


def _get_runner(reps=1, flags=(False, False, False)):
    key = f"runner{reps}_{flags}"
    if key not in _NC_CACHE:
        _NC_CACHE[key] = _make_runner(reps, flags)
    return _NC_CACHE[key]


LAST_BENCH = None


def kernel(**inputs):
    global LAST_BENCH
    import time

    import jax

    in_maps, flags = _host_prep(inputs)
    make_fn, in_names, out_names, zero_outs, sharding = _get_runner(1, flags)
    concat_in = [
        np.concatenate([np.asarray(in_maps[c][n]) for c in range(B)], axis=0)
        for n in in_names
    ]
    concat_zeros = [
        np.zeros((B * z.shape[0], *z.shape[1:]), z.dtype) for z in zero_outs
    ]
    fn1 = make_fn(1)
    dev_in = [jax.device_put(a, sharding) for a in concat_in]
    outs = fn1(*dev_in, *concat_zeros)
    jax.block_until_ready(outs)
    result = np.asarray(outs[0]).reshape(B, N, C).astype(np.float32)

    iters = int(os.environ.get("BENCH_ITERS", "0"))
    if iters > 0:
        o = fn1(*dev_in, *outs)  # warm
        jax.block_until_ready(o)
        times = []
        for _ in range(iters):
            t0 = time.perf_counter()
            o = fn1(*dev_in, *o)
            jax.block_until_ready(o)
            times.append(time.perf_counter() - t0)
        overhead = _bench_overhead()
        t_min = float(np.min(times))
        t_med = float(np.median(times))
        LAST_BENCH = {
            "per_iter_ns": max(t_min - overhead, 0.0) * 1e9,
            "call_min_ns": t_min * 1e9,
            "call_med_ns": t_med * 1e9,
            "overhead_ns": overhead * 1e9,
            "iters": iters,
        }
    return result


def _bench_overhead():
    """Per-call dispatch overhead, measured with a trivial 1-DMA kernel."""
    import time

    import jax
    from jax.experimental.shard_map import shard_map
    from jax.sharding import Mesh, PartitionSpec

    import concourse.bacc as bacc
    import concourse.tile as tile
    from concourse import bass2jax, mybir

    if "tiny" not in _NC_CACHE:
        f32 = mybir.dt.float32
        nc = bacc.Bacc(
            "TRN2",
            target_bir_lowering=False,
            debug=False,
            enable_asserts=False,
            num_devices=8,
        )
        xi = nc.dram_tensor("ti", [128, 128], f32, kind="ExternalInput").ap()
        xo = nc.dram_tensor("to", [128, 128], f32, kind="ExternalOutput").ap()
        with tile.TileContext(nc) as tc:
            with tc.tile_pool(name="p", bufs=1) as p:
                t = p.tile([128, 128], f32, tag="t", name="t")
                nc.sync.dma_start(t[:], xi[:])
                nc.sync.dma_start(xo[:], t[:])
        nc.compile()

        partition_name = nc.partition_id_tensor.name if nc.partition_id_tensor else None
        all_names = ["ti", "to"]
        if partition_name is not None:
            all_names.append(partition_name)
        out_avals = [jax.core.ShapedArray((128, 128), np.float32)]

        def _tbody(*args):
            operands = list(args)
            if partition_name is not None:
                operands.append(bass2jax.partition_id_tensor())
            return tuple(
                bass2jax._bass_exec_p.bind(
                    *operands,
                    out_avals=tuple(out_avals),
                    in_names=tuple(all_names),
                    out_names=("to",),
                    lowering_input_output_aliases=(),
                    sim_require_finite=True,
                    sim_require_nnan=True,
                    nc=nc,
                )
            )

        devices = jax.devices()[:B]
        mesh = Mesh(np.asarray(devices), ("core",))
        tfn = jax.jit(
            shard_map(
                _tbody,
                mesh=mesh,
                in_specs=(PartitionSpec("core"),) * 2,
                out_specs=(PartitionSpec("core"),),
                check_rep=False,
            ),
            donate_argnums=(1,),
            keep_unused=True,
        )
        _NC_CACHE["tiny"] = tfn

    tfn = _NC_CACHE["tiny"]
    ti = np.zeros((B * 128, 128), np.float32)
    o = tfn(ti, np.zeros((B * 128, 128), np.float32))
    jax.block_until_ready(o)
    times = []
    for _ in range(30):
        t0 = time.perf_counter()
        o = tfn(ti, *([o] if not isinstance(o, tuple) else list(o)))
        jax.block_until_ready(o)
        times.append(time.perf_counter() - t0)
    return float(np.min(times))
